# revision 25
# baseline (speedup 1.0000x reference)
"""DepthAugmentation Trainium2 kernel (v2: bf16 + packed buffers).

Reference pipeline (B=64, H=480, W=640, all f32):
  1. noise = bilinear_upsample(noise_lo * sigma, 4x)   (half-pixel centers)
     depth = clip(depth + noise * (depth > 0), 0, 1)
  2. depth *= (dropout_u >= P_DROPOUT)
  3. depth = where(random_u < P_RANDOM, random_vals, depth)
  4. per-sample stick rectangle painted with the (post-step-3) anchor value.

Sharding: pure data parallel, 8 samples per core on 8 NeuronCores.

v2 changes vs v1:
  - All big tensors ship as bf16 (half the HBM + host-relay traffic).
    Threshold tensors ship SHIFTED: dus = bf16(dropout_u - P), compared
    vs 0 on device. Sign is preserved exactly through the f32->bf16
    rounding (|du - P| >= ~2^-32 >> bf16 min normal), so the dropout /
    random masks are bit-identical to the f32 reference's.
  - All inputs packed into ONE bf16 blob + ONE i32 meta tensor per core
    (the per-exec host-relay cost scales with buffer count).
  - Output is bf16, upcast to f32 on the host (max abs err ~2^-9, well
    inside the 2e-2 gate).
  - Single bf16 matmul path for the 4x bilinear upsample (no hi/lo
    split): upsample weights {.125,.375,.625,.875,1} are exact in bf16,
    sigma is folded into noise_lo on the host.
"""

import numpy as np
import ml_dtypes

import concourse.bass as bass
import concourse.tile as tile
from concourse import mybir
from concourse.bass_utils import run_bass_kernel_spmd

F32 = mybir.dt.float32
BF16 = mybir.dt.bfloat16
FP8 = mybir.dt.float8e5
I32 = mybir.dt.int32
U8 = mybir.dt.uint8
OP = mybir.AluOpType

NOISE_SIGMA = 0.005
P_DROPOUT = 0.003125
P_RANDOM = 0.003125
P_STICK = 0.00025

B, H, W = 64, 480, 640
HL, WL = 120, 160          # noise_lo spatial dims
N_CORES = 8
SPC = B // N_CORES         # samples per core
RPC = SPC * H              # output rows per core block (3840)
IC = 4                     # rows per partition group
ICH = H // IC              # 120
PAD_IDX = 1 << 30          # OOB sentinel for indirect DMA padding
N_PROW = 256               # painted-row slots (2 chunks of 128)

# ---- blob layout (bf16 elements) ----
PIX = H * W                       # 307200
# per sample: depth(bf16)|z8(fp8e5, half the slots)|rus(bf16)|rv(bf16),
# partition-major so each partition line is one contiguous 17920B run
SAMP = 3 * PIX + PIX // 2         # 1075200
NL_OFF = SPC * SAMP               # 8601600
NL_N = HL * SPC * WL              # 153600 ([120, 8*160] layout)
AVT_OFF = NL_OFF + NL_N
AVT_N = HL * H                    # 57600
AHT_OFF = AVT_OFF + AVT_N
AHT_N = WL * W                    # 102400
FBV_OFF = AHT_OFF + AHT_N
BLOB_N = FBV_OFF + SPC            # 10144008

# ---- meta layout (i32 elements) ----
M_AIDX = 0                        # [SPC] anchor element index into out
M_PROW = M_AIDX + SPC             # [N_PROW] painted row gather index
M_SPROW = M_PROW + N_PROW         # [N_PROW] sample of each painted row
M_PXLO = M_SPROW + N_PROW         # [N_PROW] stick col start
M_PXHI = M_PXLO + N_PROW          # [N_PROW] stick col end
META_N = M_PXHI + N_PROW          # 1032


def _upsample_matrix(n_out, n_in):
    """Bilinear upsample matrix, half-pixel centers, edge clamp."""
    A = np.zeros((n_out, n_in), dtype=np.float64)
    scale = n_in / n_out
    for i in range(n_out):
        src = (i + 0.5) * scale - 0.5
        k0 = int(np.floor(src))
        f = src - k0
        A[i, min(max(k0, 0), n_in - 1)] += 1.0 - f
        A[i, min(max(k0 + 1, 0), n_in - 1)] += f
    return A.astype(np.float32)


def _split_multiwaits(nc):
    """This container's walrus build only accepts ONE sync-wait command per
    CTRL instruction; Tile's epilogue drain carries several. Hoist extra
    waits onto single-wait drains inserted just before the offender."""
    for b in nc.m.functions[0].blocks:
        insts = b.instructions
        i = 0
        while i < len(insts):
            inst = insts[i]
            si = inst.sync_info
            if si is not None and si.on_wait is not None and len(si.on_wait) > 1:
                ws = list(si.on_wait)
                while si.on_wait:
                    si.on_wait.pop()
                si.on_wait.append(ws[-1])
                for k, w in enumerate(ws[:-1]):
                    nd = mybir.InstDrain(
                        name=f"{inst.name}-wsplit{k}", ins=[], outs=[]
                    )
                    nd.engine = inst.engine
                    nd.sync_info = mybir.SyncInfo(on_wait=[w], on_update=[])
                    insts.insert(i, nd)
                    nc.inst_map[nd.name] = nd
                    i += 1
            i += 1


def _build_bass():
    nc = bass.Bass(trn_type="TRN2")

    blob = nc.dram_tensor("blob", [BLOB_N, 1], BF16, kind="ExternalInput")
    meta = nc.dram_tensor("meta", [META_N, 1], I32, kind="ExternalInput")
    out_dr = nc.dram_tensor("out", [RPC, W], BF16, kind="ExternalOutput")
    out_flat = out_dr[:].rearrange("a b -> (a b)").unsqueeze(1)

    def bslice(off, n, p):
        """blob[off:off+n] as a [p, n/p] tile AP (row-major fill)."""
        return blob[off:off + n, 0:1].rearrange("(p c) u -> p (c u)", p=p)

    with tile.TileContext(nc) as tc:
        with (
            tc.tile_pool(name="const", bufs=1) as constp,
            tc.tile_pool(name="big", bufs=4) as big_p,
            tc.tile_pool(name="u1", bufs=2) as u1_p,
            tc.tile_pool(name="q", bufs=3) as q_p,
            tc.tile_pool(name="w", bufs=6) as w_p,
            tc.tile_pool(name="rm", bufs=3) as rm_p,
            tc.tile_pool(name="stick", bufs=1) as stick_p,
            tc.tile_pool(name="ps1", bufs=2, space="PSUM") as ps1_p,
            tc.tile_pool(name="ps320", bufs=3, space="PSUM") as ps320_p,
            tc.tile_pool(name="dscr", bufs=1, space="DRAM") as dram_p,
        ):
            # ---- constants / small inputs
            nl_t = constp.tile([HL, SPC * WL], BF16)      # (120, 1280)
            nc.sync.dma_start(out=nl_t[:], in_=bslice(NL_OFF, NL_N, HL))
            avt_t = constp.tile([HL, H], BF16)            # (120, 480)
            nc.sync.dma_start(out=avt_t[:], in_=bslice(AVT_OFF, AVT_N, HL))
            aht_t0 = constp.tile([80, W], BF16)           # AhT rows 0:80
            aht_t1 = constp.tile([80, W], BF16)           # AhT rows 80:160
            nc.sync.dma_start(out=aht_t0[:], in_=bslice(AHT_OFF, 80 * W, 80))
            nc.sync.dma_start(out=aht_t1[:], in_=bslice(AHT_OFF + 80 * W, 80 * W, 80))
            fbv_t = stick_p.tile([SPC, 1], BF16)
            nc.sync.dma_start(out=fbv_t[:], in_=bslice(FBV_OFF, SPC, SPC))

            identf = constp.tile([ICH, ICH], F32)
            from concourse.masks import make_identity
            make_identity(nc, identf[:])
            ident = constp.tile([ICH, ICH], BF16)
            nc.vector.tensor_copy(ident[:], identf[:])

            colidx_i = constp.tile([128, W], I32)
            nc.gpsimd.iota(colidx_i[:], pattern=[[1, W]], base=0, channel_multiplier=0)
            colidx = constp.tile([128, W], F32)
            nc.vector.tensor_copy(colidx[:], colidx_i[:])

            # stick meta
            nch = N_PROW // 128
            aidx_t = stick_p.tile([SPC, 1], I32)
            nc.sync.dma_start(out=aidx_t[:], in_=meta[M_AIDX:M_AIDX + SPC, :])
            mt2 = lambda off: meta[off:off + N_PROW, 0:1].rearrange(
                "(c p) u -> p (c u)", c=nch
            )
            prow_t = stick_p.tile([128, nch], I32)
            nc.sync.dma_start(out=prow_t[:], in_=mt2(M_PROW))
            sprow_t = stick_p.tile([128, nch], I32)
            nc.sync.dma_start(out=sprow_t[:], in_=mt2(M_SPROW))
            pxlo_i = stick_p.tile([128, nch], I32)
            nc.sync.dma_start(out=pxlo_i[:], in_=mt2(M_PXLO))
            pxhi_i = stick_p.tile([128, nch], I32)
            nc.sync.dma_start(out=pxhi_i[:], in_=mt2(M_PXHI))
            pxlo_t = stick_p.tile([128, nch], F32)
            nc.vector.tensor_copy(pxlo_t[:], pxlo_i[:])
            pxhi_t = stick_p.tile([128, nch], F32)
            nc.vector.tensor_copy(pxhi_t[:], pxhi_i[:])

            out_dmas = []
            pending_out = []

            def flush_out():
                while pending_out:
                    qq, rr0, hh = pending_out.pop(0)
                    dma = nc.scalar.dma_start(
                        out=out_dr[rr0:rr0 + H, :].rearrange(
                            "(p g r) j -> p g r j", g=2, r=2
                        )[:, hh],
                        in_=qq[:, 1280 * hh:1280 * hh + 1280].rearrange(
                            "p (r j) -> p r j", r=2
                        ),
                    )
                    out_dmas.append(dma)

            for s in range(SPC):
                r0 = s * H
                # sample s inputs, split so d0 (which gates the PE stage)
                # lands first
                CW = IC * W                   # 2560 bf16 cols per tensor
                PCOLS = SAMP // ICH           # 8960 bf16 cols per partition
                big = big_p.tile([ICH, PCOLS], BF16)
                blob_s = blob[s * SAMP:(s + 1) * SAMP, 0:1].rearrange(
                    "(p c) u -> p (c u)", p=ICH
                )
                nc.sync.dma_start(out=big[:, 0:CW], in_=blob_s[:, 0:CW])
                nc.sync.dma_start(out=big[:, CW:PCOLS], in_=blob_s[:, CW:PCOLS])
                d0 = big[:, 0:CW]
                # z gate ships as fp8e5 (sign-exact), packed in bf16 slots
                dus = big[:, CW:CW + CW // 2].bitcast(FP8)
                rus = big[:, CW + CW // 2:2 * CW + CW // 2]
                rv = big[:, 2 * CW + CW // 2:3 * CW + CW // 2]

                # ---- stage 1: vertical upsample U1T = N^T @ Av^T
                u1 = []
                for c in range(2):
                    ps1 = ps1_p.tile([80, H], F32)
                    nc.tensor.matmul(
                        ps1[:],
                        lhsT=nl_t[:, s * WL + 80 * c:s * WL + 80 * (c + 1)],
                        rhs=avt_t[:], start=True, stop=True,
                    )
                    uc = u1_p.tile([80, H], BF16, tag=f"u1c{c}")
                    nc.scalar.copy(uc[:], ps1[:])
                    u1.append(uc)
                u1r = [u[:].rearrange("c (i r) -> c r i", r=IC) for u in u1]

                # ---- stage 2: horizontal upsample + d0 accumulate -> clip
                q = q_p.tile([ICH, IC * W], BF16)
                rm = rm_p.tile([ICH, IC * W], U8, tag="rm")
                for rho in range(4):
                    rb = rho * W
                    psA = ps320_p.tile([ICH, 320], F32, tag="psA")
                    psB = ps320_p.tile([ICH, 320], F32, tag="psB")
                    # weight-grouped matmul order: consecutive matmuls share
                    # the stationary tensor
                    for wt, rhsA, rhsB, fst, lst in (
                        (u1r[0][:, rho, :], aht_t0[:, 0:320],
                         aht_t0[:, 320:640], True, False),
                        (u1r[1][:, rho, :], aht_t1[:, 0:320],
                         aht_t1[:, 320:640], False, False),
                        (ident[:], d0[:, rb:rb + 320],
                         d0[:, rb + 320:rb + 640], False, True),
                    ):
                        nc.tensor.matmul(psA[:], lhsT=wt, rhs=rhsA,
                                         start=fst, stop=lst)
                        nc.tensor.matmul(psB[:], lhsT=wt, rhs=rhsB,
                                         start=fst, stop=lst)
                    # clip01, split across ACT (relu(1-relu(1-s))) and
                    # DVE (dual-op min/max) to balance engine load
                    w = w_p.tile([ICH, 320], F32, tag="w")
                    nc.scalar.activation(
                        out=w[:], in_=psA[:],
                        func=mybir.ActivationFunctionType.Relu,
                        scale=-1.0, bias=1.0,
                    )
                    nc.scalar.activation(
                        out=q[:, rb:rb + 320], in_=w[:],
                        func=mybir.ActivationFunctionType.Relu,
                        scale=-1.0, bias=1.0,
                    )
                    nc.vector.tensor_scalar(
                        out=q[:, rb + 320:rb + 640], in0=psB[:],
                        scalar1=0.0, scalar2=1.0, op0=OP.max, op1=OP.min,
                    )
                    if rho % 2 == 0:
                        continue
                    # ---- per-half mask chain on (120, 1280)
                    qsl = slice(rb - W, rb + W)
                    # dus ships as z = min(depth*2^30 - 1, dropout_u - P):
                    # z >= 0  <=>  (depth > 0) AND (dropout_u >= P), so one
                    # stt applies both the validity and dropout gates
                    nc.vector.scalar_tensor_tensor(
                        out=q[:, qsl], in0=dus[:, qsl], scalar=0.0,
                        in1=q[:, qsl], op0=OP.is_ge, op1=OP.mult,
                    )
                    nc.vector.tensor_scalar(
                        out=rm[:, qsl], in0=rus[:, qsl], scalar1=0.0,
                        scalar2=None, op0=OP.is_lt,
                    )
                    nc.vector.copy_predicated(
                        out=q[:, qsl], mask=rm[:, qsl], data=rv[:, qsl]
                    )
                    # defer each half's out DMA so the ACT sequencer never
                    # stalls waiting for this sample's DVE chain
                    flush_out()
                    pending_out.append((q, r0, rho // 2))
            flush_out()

            # ---- stick tail ----
            anch = stick_p.tile([SPC, 1], BF16)
            ga = nc.gpsimd.indirect_dma_start(
                out=anch[:], out_offset=None, in_=out_flat,
                in_offset=bass.IndirectOffsetOnAxis(ap=aidx_t[:, :1], axis=0),
            )
            for d in out_dmas:
                tile.add_dep_helper(ga.ins, d.ins)
            m8 = stick_p.tile([SPC, 1], U8)
            nc.vector.tensor_scalar(
                out=m8[:], in0=anch[:], scalar1=0.0, scalar2=None, op0=OP.is_gt
            )
            val = stick_p.tile([SPC, 1], BF16)
            nc.vector.tensor_copy(val[:], fbv_t[:])
            nc.vector.copy_predicated(out=val[:], mask=m8[:], data=anch[:])
            vscr = dram_p.tile([SPC, 1], BF16)
            nc.sync.dma_start(out=vscr[:], in_=val[:])

            for ch in range(nch):
                vrow = stick_p.tile([128, 1], BF16, tag=f"vrow{ch}")
                nc.gpsimd.indirect_dma_start(
                    out=vrow[:], out_offset=None, in_=vscr[:],
                    in_offset=bass.IndirectOffsetOnAxis(
                        ap=sprow_t[:, ch:ch + 1], axis=0
                    ),
                )
                g = stick_p.tile([128, W], BF16, tag=f"g{ch}")
                nc.vector.memset(g[:], 0.0)
                gr = nc.gpsimd.indirect_dma_start(
                    out=g[:], out_offset=None, in_=out_dr[:],
                    in_offset=bass.IndirectOffsetOnAxis(
                        ap=prow_t[:, ch:ch + 1], axis=0
                    ),
                    bounds_check=RPC - 1, oob_is_err=False,
                )
                for d in out_dmas:
                    tile.add_dep_helper(gr.ins, d.ins)
                cm1 = stick_p.tile([128, W], U8, tag=f"cm1{ch}")
                nc.vector.tensor_scalar(
                    out=cm1[:], in0=colidx[:], scalar1=pxlo_t[:, ch:ch + 1],
                    scalar2=None, op0=OP.is_ge,
                )
                cm = stick_p.tile([128, W], U8, tag=f"cm{ch}")
                nc.vector.scalar_tensor_tensor(
                    out=cm[:], in0=colidx[:], scalar=pxhi_t[:, ch:ch + 1],
                    in1=cm1[:], op0=OP.is_lt, op1=OP.mult,
                )
                nc.vector.copy_predicated(
                    out=g[:], mask=cm[:], data=vrow[:].to_broadcast([128, W])
                )
                nc.gpsimd.indirect_dma_start(
                    out=out_dr[:],
                    out_offset=bass.IndirectOffsetOnAxis(
                        ap=prow_t[:, ch:ch + 1], axis=0
                    ),
                    in_=g[:], in_offset=None,
                    bounds_check=RPC - 1, oob_is_err=False,
                )

    _split_multiwaits(nc)
    return nc


def _stick_params(stick_len, stick_width, stick_y, stick_x, horiz_u, stick_u):
    """Vectorized reference stick geometry (ints, host side)."""
    length = stick_len.astype(np.int64) + 1
    width = stick_width.astype(np.int64) + 1
    horiz = horiz_u > 0.5
    span_h = np.where(horiz, width, length)
    span_w = np.where(horiz, length, width)
    y = np.clip(stick_y.astype(np.int64), 0, np.maximum(H - span_h, 1) - 1)
    x = np.clip(stick_x.astype(np.int64), 0, np.maximum(W - span_w, 1) - 1)
    stick_on = stick_u < np.float32(P_STICK * H * W)
    return y, x, span_h, span_w, stick_on


def _to_bf16(a):
    return np.asarray(a, dtype=np.float32).astype(ml_dtypes.bfloat16)


_NC_CACHE = []


def kernel(**inputs):
    depth = np.asarray(inputs["depth"], dtype=np.float32)
    noise_lo = np.asarray(inputs["noise_lo"], dtype=np.float32)
    dropout_u = np.asarray(inputs["dropout_u"], dtype=np.float32)
    random_u = np.asarray(inputs["random_u"], dtype=np.float32)
    random_vals = np.asarray(inputs["random_vals"], dtype=np.float32)
    stick_u = np.asarray(inputs["stick_u"], dtype=np.float32)
    horiz_u = np.asarray(inputs["horiz_u"], dtype=np.float32)
    fallback_vals = np.asarray(inputs["fallback_vals"], dtype=np.float32)
    stick_len = np.asarray(inputs["stick_len"])
    stick_width = np.asarray(inputs["stick_width"])
    stick_y = np.asarray(inputs["stick_y"])
    stick_x = np.asarray(inputs["stick_x"])

    T32 = np.float32(P_DROPOUT)
    avt = _to_bf16(_upsample_matrix(H, HL).T)         # (120, 480)
    aht = _to_bf16(_upsample_matrix(W, WL).T)         # (160, 640)

    y, x, span_h, span_w, stick_on = _stick_params(
        stick_len, stick_width, stick_y, stick_x, horiz_u, stick_u
    )

    depth_b = _to_bf16(depth).reshape(B, ICH, IC * W)
    # z < 0 iff depth == 0 or dropout_u < P (branch-free sign encoding;
    # nonzero uniform f32 depths are >= 2^-23, so depth*2^30 - 1 > 0).
    # Shipped as fp8e5m2 after scaling by 2^40: |z| >= ~2^-32 so the scaled
    # magnitude is >= 2^8, far above fp8 underflow; overflow saturates to
    # +-inf with the sign intact. The device only tests z >= 0.
    z8_b = (
        np.minimum(depth * np.float32(2.0 ** 30) - np.float32(1.0),
                   dropout_u - T32) * np.float32(2.0 ** 40)
    ).astype(ml_dtypes.float8_e5m2).view(np.uint8).reshape(B, ICH, IC * W)
    rus_b = _to_bf16(random_u - T32).reshape(B, ICH, IC * W)
    rv_b = _to_bf16(random_vals).reshape(B, ICH, IC * W)
    # noise_lo * sigma, transposed per core to [120, SPC*160]
    nl_s = (noise_lo[:, 0] * np.float32(NOISE_SIGMA)).astype(ml_dtypes.bfloat16)
    fbv_b = _to_bf16(fallback_vals)

    in_maps = []
    for k in range(N_CORES):
        s0 = k * SPC
        sl = slice(s0, s0 + SPC)
        blob = np.empty((BLOB_N, 1), ml_dtypes.bfloat16)
        # partition-major byte layout per [sample][partition]:
        # depth 5120B | z8 2560B | rus 5120B | rv 5120B
        bigv = blob[:NL_OFF, 0].view(np.uint8).reshape(SPC, ICH, 17920)
        bigv[:, :, 0:5120] = depth_b[sl].view(np.uint8)
        bigv[:, :, 5120:7680] = z8_b[sl]
        bigv[:, :, 7680:12800] = rus_b[sl].view(np.uint8)
        bigv[:, :, 12800:17920] = rv_b[sl].view(np.uint8)
        # [SPC, 120, 160] -> [120, SPC, 160]
        blob[NL_OFF:NL_OFF + NL_N, 0] = np.ascontiguousarray(
            nl_s[sl].transpose(1, 0, 2)
        ).reshape(-1)
        blob[AVT_OFF:AVT_OFF + AVT_N, 0] = avt.reshape(-1)
        blob[AHT_OFF:AHT_OFF + AHT_N, 0] = aht.reshape(-1)
        blob[FBV_OFF:FBV_OFF + SPC, 0] = fbv_b[sl]

        metav = np.zeros((META_N, 1), np.int32)
        prow = metav[M_PROW:M_PROW + N_PROW, 0]
        prow[:] = PAD_IDX
        sprow = metav[M_SPROW:M_SPROW + N_PROW, 0]
        pxlo = metav[M_PXLO:M_PXLO + N_PROW, 0]
        pxhi = metav[M_PXHI:M_PXHI + N_PROW, 0]
        aidx = metav[M_AIDX:M_AIDX + SPC, 0]
        n = 0
        for s in range(SPC):
            b = s0 + s
            aidx[s] = (s * H + y[b]) * W + x[b]
            if not stick_on[b]:
                continue
            for r in range(int(span_h[b])):
                prow[n] = s * H + y[b] + r
                sprow[n] = s
                pxlo[n] = x[b]
                pxhi[n] = x[b] + span_w[b]
                n += 1
        in_maps.append({"blob": blob, "meta": metav})

    if not _NC_CACHE:
        _NC_CACHE.append(_build_bass())
    nc = _NC_CACHE[0]
    res = run_bass_kernel_spmd(nc, in_maps, core_ids=list(range(N_CORES)))
    out = np.empty((B, 1, H, W), np.float32)
    for k in range(N_CORES):
        out[k * SPC:(k + 1) * SPC, 0] = (
            res.results[k]["out"].astype(np.float32).reshape(SPC, H, W)
        )
    return out


# revision 26
# speedup vs baseline: 1.1081x; 1.1081x over previous
"""DepthAugmentation Trainium2 kernel (v2: bf16 + packed buffers).

Reference pipeline (B=64, H=480, W=640, all f32):
  1. noise = bilinear_upsample(noise_lo * sigma, 4x)   (half-pixel centers)
     depth = clip(depth + noise * (depth > 0), 0, 1)
  2. depth *= (dropout_u >= P_DROPOUT)
  3. depth = where(random_u < P_RANDOM, random_vals, depth)
  4. per-sample stick rectangle painted with the (post-step-3) anchor value.

Sharding: pure data parallel, 8 samples per core on 8 NeuronCores.

v2 changes vs v1:
  - All big tensors ship as bf16 (half the HBM + host-relay traffic).
    Threshold tensors ship SHIFTED: dus = bf16(dropout_u - P), compared
    vs 0 on device. Sign is preserved exactly through the f32->bf16
    rounding (|du - P| >= ~2^-32 >> bf16 min normal), so the dropout /
    random masks are bit-identical to the f32 reference's.
  - All inputs packed into ONE bf16 blob + ONE i32 meta tensor per core
    (the per-exec host-relay cost scales with buffer count).
  - Output is bf16, upcast to f32 on the host (max abs err ~2^-9, well
    inside the 2e-2 gate).
  - Single bf16 matmul path for the 4x bilinear upsample (no hi/lo
    split): upsample weights {.125,.375,.625,.875,1} are exact in bf16,
    sigma is folded into noise_lo on the host.
"""

import numpy as np
import ml_dtypes

import concourse.bass as bass
import concourse.tile as tile
from concourse import mybir
from concourse.bass_utils import run_bass_kernel_spmd

F32 = mybir.dt.float32
BF16 = mybir.dt.bfloat16
FP8 = mybir.dt.float8e5
I32 = mybir.dt.int32
U8 = mybir.dt.uint8
OP = mybir.AluOpType

NOISE_SIGMA = 0.005
P_DROPOUT = 0.003125
P_RANDOM = 0.003125
P_STICK = 0.00025

B, H, W = 64, 480, 640
HL, WL = 120, 160          # noise_lo spatial dims
N_CORES = 8
SPC = B // N_CORES         # samples per core
RPC = SPC * H              # output rows per core block (3840)
IC = 4                     # rows per partition group
ICH = H // IC              # 120
PAD_IDX = 1 << 30          # OOB sentinel for indirect DMA padding
N_PROW = 256               # painted-row slots (2 chunks of 128)

# ---- blob layout (bf16 elements) ----
PIX = H * W                       # 307200
# per sample: depth(bf16)|z8(fp8e5, half the slots)|rus(bf16)|rv(bf16),
# partition-major so each partition line is one contiguous 17920B run
SAMP = 3 * PIX + PIX // 2         # 1075200
NL_OFF = SPC * SAMP               # 8601600
NL_N = HL * SPC * WL              # 153600 ([120, 8*160] layout)
AVT_OFF = NL_OFF + NL_N
AVT_N = HL * H                    # 57600
AHT_OFF = AVT_OFF + AVT_N
AHT_N = WL * W                    # 102400
FBV_OFF = AHT_OFF + AHT_N
BLOB_N = FBV_OFF + SPC            # 10144008

# ---- meta layout (i32 elements) ----
M_AIDX = 0                        # [SPC] anchor element index into out
M_PROW = M_AIDX + SPC             # [N_PROW] painted row gather index
M_SPROW = M_PROW + N_PROW         # [N_PROW] sample of each painted row
M_PXLO = M_SPROW + N_PROW         # [N_PROW] stick col start
M_PXHI = M_PXLO + N_PROW          # [N_PROW] stick col end
META_N = M_PXHI + N_PROW          # 1032


def _upsample_matrix(n_out, n_in):
    """Bilinear upsample matrix, half-pixel centers, edge clamp."""
    A = np.zeros((n_out, n_in), dtype=np.float64)
    scale = n_in / n_out
    for i in range(n_out):
        src = (i + 0.5) * scale - 0.5
        k0 = int(np.floor(src))
        f = src - k0
        A[i, min(max(k0, 0), n_in - 1)] += 1.0 - f
        A[i, min(max(k0 + 1, 0), n_in - 1)] += f
    return A.astype(np.float32)


def _split_multiwaits(nc):
    """This container's walrus build only accepts ONE sync-wait command per
    CTRL instruction; Tile's epilogue drain carries several. Hoist extra
    waits onto single-wait drains inserted just before the offender."""
    for b in nc.m.functions[0].blocks:
        insts = b.instructions
        i = 0
        while i < len(insts):
            inst = insts[i]
            si = inst.sync_info
            if si is not None and si.on_wait is not None and len(si.on_wait) > 1:
                ws = list(si.on_wait)
                while si.on_wait:
                    si.on_wait.pop()
                si.on_wait.append(ws[-1])
                for k, w in enumerate(ws[:-1]):
                    nd = mybir.InstDrain(
                        name=f"{inst.name}-wsplit{k}", ins=[], outs=[]
                    )
                    nd.engine = inst.engine
                    nd.sync_info = mybir.SyncInfo(on_wait=[w], on_update=[])
                    insts.insert(i, nd)
                    nc.inst_map[nd.name] = nd
                    i += 1
            i += 1


def _build_bass():
    nc = bass.Bass(trn_type="TRN2")

    blob = nc.dram_tensor("blob", [BLOB_N, 1], BF16, kind="ExternalInput")
    meta = nc.dram_tensor("meta", [META_N, 1], I32, kind="ExternalInput")
    out_dr = nc.dram_tensor("out", [RPC, W], BF16, kind="ExternalOutput")
    out_flat = out_dr[:].rearrange("a b -> (a b)").unsqueeze(1)

    def bslice(off, n, p):
        """blob[off:off+n] as a [p, n/p] tile AP (row-major fill)."""
        return blob[off:off + n, 0:1].rearrange("(p c) u -> p (c u)", p=p)

    with tile.TileContext(nc) as tc:
        with (
            tc.tile_pool(name="const", bufs=1) as constp,
            tc.tile_pool(name="big", bufs=4) as big_p,
            tc.tile_pool(name="u1", bufs=2) as u1_p,
            tc.tile_pool(name="q", bufs=3) as q_p,
            tc.tile_pool(name="w", bufs=6) as w_p,
            tc.tile_pool(name="rm", bufs=3) as rm_p,
            tc.tile_pool(name="stick", bufs=1) as stick_p,
            tc.tile_pool(name="ps1", bufs=2, space="PSUM") as ps1_p,
            tc.tile_pool(name="ps320", bufs=3, space="PSUM") as ps320_p,
            tc.tile_pool(name="dscr", bufs=1, space="DRAM") as dram_p,
        ):
            # ---- constants / small inputs
            nl_t = constp.tile([HL, SPC * WL], BF16)      # (120, 1280)
            nc.sync.dma_start(out=nl_t[:], in_=bslice(NL_OFF, NL_N, HL))
            avt_t = constp.tile([HL, H], BF16)            # (120, 480)
            nc.sync.dma_start(out=avt_t[:], in_=bslice(AVT_OFF, AVT_N, HL))
            aht_t0 = constp.tile([80, W], BF16)           # AhT rows 0:80
            aht_t1 = constp.tile([80, W], BF16)           # AhT rows 80:160
            nc.sync.dma_start(out=aht_t0[:], in_=bslice(AHT_OFF, 80 * W, 80))
            nc.sync.dma_start(out=aht_t1[:], in_=bslice(AHT_OFF + 80 * W, 80 * W, 80))
            fbv_t = stick_p.tile([SPC, 1], BF16)
            nc.sync.dma_start(out=fbv_t[:], in_=bslice(FBV_OFF, SPC, SPC))

            identf = constp.tile([ICH, ICH], F32)
            from concourse.masks import make_identity
            make_identity(nc, identf[:])
            ident = constp.tile([ICH, ICH], BF16)
            nc.vector.tensor_copy(ident[:], identf[:])

            colidx_i = constp.tile([128, W], I32)
            nc.gpsimd.iota(colidx_i[:], pattern=[[1, W]], base=0, channel_multiplier=0)
            colidx = constp.tile([128, W], F32)
            nc.vector.tensor_copy(colidx[:], colidx_i[:])

            # stick meta
            nch = N_PROW // 128
            aidx_t = stick_p.tile([SPC, 1], I32)
            nc.sync.dma_start(out=aidx_t[:], in_=meta[M_AIDX:M_AIDX + SPC, :])
            mt2 = lambda off: meta[off:off + N_PROW, 0:1].rearrange(
                "(c p) u -> p (c u)", c=nch
            )
            prow_t = stick_p.tile([128, nch], I32)
            nc.sync.dma_start(out=prow_t[:], in_=mt2(M_PROW))
            sprow_t = stick_p.tile([128, nch], I32)
            nc.sync.dma_start(out=sprow_t[:], in_=mt2(M_SPROW))
            pxlo_i = stick_p.tile([128, nch], I32)
            nc.sync.dma_start(out=pxlo_i[:], in_=mt2(M_PXLO))
            pxhi_i = stick_p.tile([128, nch], I32)
            nc.sync.dma_start(out=pxhi_i[:], in_=mt2(M_PXHI))
            pxlo_t = stick_p.tile([128, nch], F32)
            nc.vector.tensor_copy(pxlo_t[:], pxlo_i[:])
            pxhi_t = stick_p.tile([128, nch], F32)
            nc.vector.tensor_copy(pxhi_t[:], pxhi_i[:])

            out_dmas = []
            pending_out = []

            def flush_out():
                while pending_out:
                    qq, rr0, hh = pending_out.pop(0)
                    dma = nc.scalar.dma_start(
                        out=out_dr[rr0:rr0 + H, :].rearrange(
                            "(p g r) j -> p g r j", g=2, r=2
                        )[:, hh],
                        in_=qq[:, 1280 * hh:1280 * hh + 1280].rearrange(
                            "p (r j) -> p r j", r=2
                        ),
                    )
                    out_dmas.append(dma)

            for s in range(SPC):
                r0 = s * H
                # sample s inputs, split so d0 (which gates the PE stage)
                # lands first
                CW = IC * W                   # 2560 bf16 cols per tensor
                PCOLS = SAMP // ICH           # 8960 bf16 cols per partition
                big = big_p.tile([ICH, PCOLS], BF16)
                blob_s = blob[s * SAMP:(s + 1) * SAMP, 0:1].rearrange(
                    "(p c) u -> p (c u)", p=ICH
                )
                nc.sync.dma_start(out=big[:, 0:CW], in_=blob_s[:, 0:CW])
                nc.sync.dma_start(out=big[:, CW:PCOLS], in_=blob_s[:, CW:PCOLS])
                d0 = big[:, 0:CW]
                # z gate ships as fp8e5 (sign-exact), packed in bf16 slots
                dus = big[:, CW:CW + CW // 2].bitcast(FP8)
                rus = big[:, CW + CW // 2:2 * CW + CW // 2]
                rv = big[:, 2 * CW + CW // 2:3 * CW + CW // 2]

                # ---- stage 1: vertical upsample U1T = N^T @ Av^T
                u1 = []
                for c in range(2):
                    ps1 = ps1_p.tile([80, H], F32)
                    nc.tensor.matmul(
                        ps1[:],
                        lhsT=nl_t[:, s * WL + 80 * c:s * WL + 80 * (c + 1)],
                        rhs=avt_t[:], start=True, stop=True,
                    )
                    uc = u1_p.tile([80, H], BF16, tag=f"u1c{c}")
                    nc.scalar.copy(uc[:], ps1[:])
                    u1.append(uc)
                u1r = [u[:].rearrange("c (i r) -> c r i", r=IC) for u in u1]

                # ---- stage 2: horizontal upsample + d0 accumulate -> clip
                q = q_p.tile([ICH, IC * W], BF16)
                rm = rm_p.tile([ICH, IC * W], U8, tag="rm")
                for rho in range(4):
                    rb = rho * W
                    psA = ps320_p.tile([ICH, 320], F32, tag="psA")
                    psB = ps320_p.tile([ICH, 320], F32, tag="psB")
                    # weight-grouped matmul order: consecutive matmuls share
                    # the stationary tensor
                    for wt, rhsA, rhsB, fst, lst in (
                        (u1r[0][:, rho, :], aht_t0[:, 0:320],
                         aht_t0[:, 320:640], True, False),
                        (u1r[1][:, rho, :], aht_t1[:, 0:320],
                         aht_t1[:, 320:640], False, False),
                        (ident[:], d0[:, rb:rb + 320],
                         d0[:, rb + 320:rb + 640], False, True),
                    ):
                        nc.tensor.matmul(psA[:], lhsT=wt, rhs=rhsA,
                                         start=fst, stop=lst)
                        nc.tensor.matmul(psB[:], lhsT=wt, rhs=rhsB,
                                         start=fst, stop=lst)
                    # clip01, split across ACT (relu(1-relu(1-s))) and
                    # DVE (dual-op min/max) to balance engine load
                    w = w_p.tile([ICH, 320], F32, tag="w")
                    nc.scalar.activation(
                        out=w[:], in_=psA[:],
                        func=mybir.ActivationFunctionType.Relu,
                        scale=-1.0, bias=1.0,
                    )
                    nc.scalar.activation(
                        out=q[:, rb:rb + 320], in_=w[:],
                        func=mybir.ActivationFunctionType.Relu,
                        scale=-1.0, bias=1.0,
                    )
                    nc.vector.tensor_scalar(
                        out=q[:, rb + 320:rb + 640], in0=psB[:],
                        scalar1=0.0, scalar2=1.0, op0=OP.max, op1=OP.min,
                    )
                    if rho % 2 == 0:
                        continue
                    # ---- per-half mask chain on (120, 1280)
                    qsl = slice(rb - W, rb + W)
                    # dus ships as z = min(depth*2^30 - 1, dropout_u - P):
                    # z >= 0  <=>  (depth > 0) AND (dropout_u >= P), so one
                    # stt applies both the validity and dropout gates
                    nc.vector.scalar_tensor_tensor(
                        out=q[:, qsl], in0=dus[:, qsl], scalar=0.0,
                        in1=q[:, qsl], op0=OP.is_ge, op1=OP.mult,
                    )
                    nc.vector.tensor_scalar(
                        out=rm[:, qsl], in0=rus[:, qsl], scalar1=0.0,
                        scalar2=None, op0=OP.is_lt,
                    )
                    nc.vector.copy_predicated(
                        out=q[:, qsl], mask=rm[:, qsl], data=rv[:, qsl]
                    )
                    # defer each half's out DMA so the ACT sequencer never
                    # stalls waiting for this sample's DVE chain
                    flush_out()
                    pending_out.append((q, r0, rho // 2))
            flush_out()

            # ---- stick tail ----
            # column masks depend only on meta: compute them up front so the
            # post-output tail is just gather -> paint -> scatter
            cms, gs, vrows = [], [], []
            for ch in range(nch):
                cm1 = stick_p.tile([128, W], U8, tag=f"cm1{ch}")
                nc.vector.tensor_scalar(
                    out=cm1[:], in0=colidx[:], scalar1=pxlo_t[:, ch:ch + 1],
                    scalar2=None, op0=OP.is_ge,
                )
                cm = stick_p.tile([128, W], U8, tag=f"cm{ch}")
                nc.vector.scalar_tensor_tensor(
                    out=cm[:], in0=colidx[:], scalar=pxhi_t[:, ch:ch + 1],
                    in1=cm1[:], op0=OP.is_lt, op1=OP.mult,
                )
                cms.append(cm)
                g = stick_p.tile([128, W], BF16, tag=f"g{ch}")
                nc.vector.memset(g[:], 0.0)
                gs.append(g)

            anch = stick_p.tile([SPC, 1], BF16)
            ga = nc.gpsimd.indirect_dma_start(
                out=anch[:], out_offset=None, in_=out_flat,
                in_offset=bass.IndirectOffsetOnAxis(ap=aidx_t[:, :1], axis=0),
            )
            for d in out_dmas:
                tile.add_dep_helper(ga.ins, d.ins)
            # painted-row gathers are independent of the anchor/value chain
            for ch in range(nch):
                gr = nc.gpsimd.indirect_dma_start(
                    out=gs[ch][:], out_offset=None, in_=out_dr[:],
                    in_offset=bass.IndirectOffsetOnAxis(
                        ap=prow_t[:, ch:ch + 1], axis=0
                    ),
                    bounds_check=RPC - 1, oob_is_err=False,
                )
                for d in out_dmas:
                    tile.add_dep_helper(gr.ins, d.ins)
            m8 = stick_p.tile([SPC, 1], U8)
            nc.vector.tensor_scalar(
                out=m8[:], in0=anch[:], scalar1=0.0, scalar2=None, op0=OP.is_gt
            )
            val = stick_p.tile([SPC, 1], BF16)
            nc.vector.tensor_copy(val[:], fbv_t[:])
            nc.vector.copy_predicated(out=val[:], mask=m8[:], data=anch[:])
            vscr = dram_p.tile([SPC, 1], BF16)
            nc.sync.dma_start(out=vscr[:], in_=val[:])

            for ch in range(nch):
                vrow = stick_p.tile([128, 1], BF16, tag=f"vrow{ch}")
                nc.gpsimd.indirect_dma_start(
                    out=vrow[:], out_offset=None, in_=vscr[:],
                    in_offset=bass.IndirectOffsetOnAxis(
                        ap=sprow_t[:, ch:ch + 1], axis=0
                    ),
                )
                vrows.append(vrow)
            for ch in range(nch):
                nc.vector.copy_predicated(
                    out=gs[ch][:], mask=cms[ch][:],
                    data=vrows[ch][:].to_broadcast([128, W]),
                )
            for ch in range(nch):
                nc.gpsimd.indirect_dma_start(
                    out=out_dr[:],
                    out_offset=bass.IndirectOffsetOnAxis(
                        ap=prow_t[:, ch:ch + 1], axis=0
                    ),
                    in_=gs[ch][:], in_offset=None,
                    bounds_check=RPC - 1, oob_is_err=False,
                )

    _split_multiwaits(nc)
    return nc


def _stick_params(stick_len, stick_width, stick_y, stick_x, horiz_u, stick_u):
    """Vectorized reference stick geometry (ints, host side)."""
    length = stick_len.astype(np.int64) + 1
    width = stick_width.astype(np.int64) + 1
    horiz = horiz_u > 0.5
    span_h = np.where(horiz, width, length)
    span_w = np.where(horiz, length, width)
    y = np.clip(stick_y.astype(np.int64), 0, np.maximum(H - span_h, 1) - 1)
    x = np.clip(stick_x.astype(np.int64), 0, np.maximum(W - span_w, 1) - 1)
    stick_on = stick_u < np.float32(P_STICK * H * W)
    return y, x, span_h, span_w, stick_on


def _to_bf16(a):
    return np.asarray(a, dtype=np.float32).astype(ml_dtypes.bfloat16)


_NC_CACHE = []


def kernel(**inputs):
    depth = np.asarray(inputs["depth"], dtype=np.float32)
    noise_lo = np.asarray(inputs["noise_lo"], dtype=np.float32)
    dropout_u = np.asarray(inputs["dropout_u"], dtype=np.float32)
    random_u = np.asarray(inputs["random_u"], dtype=np.float32)
    random_vals = np.asarray(inputs["random_vals"], dtype=np.float32)
    stick_u = np.asarray(inputs["stick_u"], dtype=np.float32)
    horiz_u = np.asarray(inputs["horiz_u"], dtype=np.float32)
    fallback_vals = np.asarray(inputs["fallback_vals"], dtype=np.float32)
    stick_len = np.asarray(inputs["stick_len"])
    stick_width = np.asarray(inputs["stick_width"])
    stick_y = np.asarray(inputs["stick_y"])
    stick_x = np.asarray(inputs["stick_x"])

    T32 = np.float32(P_DROPOUT)
    avt = _to_bf16(_upsample_matrix(H, HL).T)         # (120, 480)
    aht = _to_bf16(_upsample_matrix(W, WL).T)         # (160, 640)

    y, x, span_h, span_w, stick_on = _stick_params(
        stick_len, stick_width, stick_y, stick_x, horiz_u, stick_u
    )

    depth_b = _to_bf16(depth).reshape(B, ICH, IC * W)
    # z < 0 iff depth == 0 or dropout_u < P (branch-free sign encoding;
    # nonzero uniform f32 depths are >= 2^-23, so depth*2^30 - 1 > 0).
    # Shipped as fp8e5m2 after scaling by 2^40: |z| >= ~2^-32 so the scaled
    # magnitude is >= 2^8, far above fp8 underflow; overflow saturates to
    # +-inf with the sign intact. The device only tests z >= 0.
    z8_b = (
        np.minimum(depth * np.float32(2.0 ** 30) - np.float32(1.0),
                   dropout_u - T32) * np.float32(2.0 ** 40)
    ).astype(ml_dtypes.float8_e5m2).view(np.uint8).reshape(B, ICH, IC * W)
    rus_b = _to_bf16(random_u - T32).reshape(B, ICH, IC * W)
    rv_b = _to_bf16(random_vals).reshape(B, ICH, IC * W)
    # noise_lo * sigma, transposed per core to [120, SPC*160]
    nl_s = (noise_lo[:, 0] * np.float32(NOISE_SIGMA)).astype(ml_dtypes.bfloat16)
    fbv_b = _to_bf16(fallback_vals)

    in_maps = []
    for k in range(N_CORES):
        s0 = k * SPC
        sl = slice(s0, s0 + SPC)
        blob = np.empty((BLOB_N, 1), ml_dtypes.bfloat16)
        # partition-major byte layout per [sample][partition]:
        # depth 5120B | z8 2560B | rus 5120B | rv 5120B
        bigv = blob[:NL_OFF, 0].view(np.uint8).reshape(SPC, ICH, 17920)
        bigv[:, :, 0:5120] = depth_b[sl].view(np.uint8)
        bigv[:, :, 5120:7680] = z8_b[sl]
        bigv[:, :, 7680:12800] = rus_b[sl].view(np.uint8)
        bigv[:, :, 12800:17920] = rv_b[sl].view(np.uint8)
        # [SPC, 120, 160] -> [120, SPC, 160]
        blob[NL_OFF:NL_OFF + NL_N, 0] = np.ascontiguousarray(
            nl_s[sl].transpose(1, 0, 2)
        ).reshape(-1)
        blob[AVT_OFF:AVT_OFF + AVT_N, 0] = avt.reshape(-1)
        blob[AHT_OFF:AHT_OFF + AHT_N, 0] = aht.reshape(-1)
        blob[FBV_OFF:FBV_OFF + SPC, 0] = fbv_b[sl]

        metav = np.zeros((META_N, 1), np.int32)
        prow = metav[M_PROW:M_PROW + N_PROW, 0]
        prow[:] = PAD_IDX
        sprow = metav[M_SPROW:M_SPROW + N_PROW, 0]
        pxlo = metav[M_PXLO:M_PXLO + N_PROW, 0]
        pxhi = metav[M_PXHI:M_PXHI + N_PROW, 0]
        aidx = metav[M_AIDX:M_AIDX + SPC, 0]
        n = 0
        for s in range(SPC):
            b = s0 + s
            aidx[s] = (s * H + y[b]) * W + x[b]
            if not stick_on[b]:
                continue
            for r in range(int(span_h[b])):
                prow[n] = s * H + y[b] + r
                sprow[n] = s
                pxlo[n] = x[b]
                pxhi[n] = x[b] + span_w[b]
                n += 1
        in_maps.append({"blob": blob, "meta": metav})

    if not _NC_CACHE:
        _NC_CACHE.append(_build_bass())
    nc = _NC_CACHE[0]
    res = run_bass_kernel_spmd(nc, in_maps, core_ids=list(range(N_CORES)))
    out = np.empty((B, 1, H, W), np.float32)
    for k in range(N_CORES):
        out[k * SPC:(k + 1) * SPC, 0] = (
            res.results[k]["out"].astype(np.float32).reshape(SPC, H, W)
        )
    return out


# revision 27
# speedup vs baseline: 1.2384x; 1.1176x over previous
"""DepthAugmentation Trainium2 kernel (v2: bf16 + packed buffers).

Reference pipeline (B=64, H=480, W=640, all f32):
  1. noise = bilinear_upsample(noise_lo * sigma, 4x)   (half-pixel centers)
     depth = clip(depth + noise * (depth > 0), 0, 1)
  2. depth *= (dropout_u >= P_DROPOUT)
  3. depth = where(random_u < P_RANDOM, random_vals, depth)
  4. per-sample stick rectangle painted with the (post-step-3) anchor value.

Sharding: pure data parallel, 8 samples per core on 8 NeuronCores.

v2 changes vs v1:
  - All big tensors ship as bf16 (half the HBM + host-relay traffic).
    Threshold tensors ship SHIFTED: dus = bf16(dropout_u - P), compared
    vs 0 on device. Sign is preserved exactly through the f32->bf16
    rounding (|du - P| >= ~2^-32 >> bf16 min normal), so the dropout /
    random masks are bit-identical to the f32 reference's.
  - All inputs packed into ONE bf16 blob + ONE i32 meta tensor per core
    (the per-exec host-relay cost scales with buffer count).
  - Output is bf16, upcast to f32 on the host (max abs err ~2^-9, well
    inside the 2e-2 gate).
  - Single bf16 matmul path for the 4x bilinear upsample (no hi/lo
    split): upsample weights {.125,.375,.625,.875,1} are exact in bf16,
    sigma is folded into noise_lo on the host.
"""

import numpy as np
import ml_dtypes

import concourse.bass as bass
import concourse.tile as tile
from concourse import mybir
from concourse.bass_utils import run_bass_kernel_spmd

F32 = mybir.dt.float32
BF16 = mybir.dt.bfloat16
FP8 = mybir.dt.float8e5
I32 = mybir.dt.int32
U8 = mybir.dt.uint8
OP = mybir.AluOpType

NOISE_SIGMA = 0.005
P_DROPOUT = 0.003125
P_RANDOM = 0.003125
P_STICK = 0.00025

B, H, W = 64, 480, 640
HL, WL = 120, 160          # noise_lo spatial dims
N_CORES = 8
SPC = B // N_CORES         # samples per core
RPC = SPC * H              # output rows per core block (3840)
IC = 4                     # rows per partition group
ICH = H // IC              # 120
PAD_IDX = 1 << 30          # OOB sentinel for indirect DMA padding
N_PROW = 256               # painted-row slots (2 chunks of 128)

# ---- blob layout (bf16 elements) ----
PIX = H * W                       # 307200
# per sample: depth(bf16)|z8(fp8e5, half the slots)|rus(bf16)|rv(bf16),
# partition-major so each partition line is one contiguous 17920B run
SAMP = 3 * PIX + PIX // 2         # 1075200
NL_OFF = SPC * SAMP               # 8601600
NL_N = HL * SPC * WL              # 153600 ([120, 8*160] layout)
AVT_OFF = NL_OFF + NL_N
AVT_N = HL * H                    # 57600
AHT_OFF = AVT_OFF + AVT_N
AHT_N = WL * W                    # 102400
FBV_OFF = AHT_OFF + AHT_N
BLOB_N = FBV_OFF + SPC            # 10144008

# ---- meta layout (i32 elements) ----
M_AIDX = 0                        # [SPC] anchor element index into out
M_PROW = M_AIDX + SPC             # [N_PROW] painted row gather index
M_SPROW = M_PROW + N_PROW         # [N_PROW] sample of each painted row
M_PXLO = M_SPROW + N_PROW         # [N_PROW] stick col start
M_PXHI = M_PXLO + N_PROW          # [N_PROW] stick col end
META_N = M_PXHI + N_PROW          # 1032


def _upsample_matrix(n_out, n_in):
    """Bilinear upsample matrix, half-pixel centers, edge clamp."""
    A = np.zeros((n_out, n_in), dtype=np.float64)
    scale = n_in / n_out
    for i in range(n_out):
        src = (i + 0.5) * scale - 0.5
        k0 = int(np.floor(src))
        f = src - k0
        A[i, min(max(k0, 0), n_in - 1)] += 1.0 - f
        A[i, min(max(k0 + 1, 0), n_in - 1)] += f
    return A.astype(np.float32)


def _split_multiwaits(nc):
    """This container's walrus build only accepts ONE sync-wait command per
    CTRL instruction; Tile's epilogue drain carries several. Hoist extra
    waits onto single-wait drains inserted just before the offender."""
    for b in nc.m.functions[0].blocks:
        insts = b.instructions
        i = 0
        while i < len(insts):
            inst = insts[i]
            si = inst.sync_info
            if si is not None and si.on_wait is not None and len(si.on_wait) > 1:
                ws = list(si.on_wait)
                while si.on_wait:
                    si.on_wait.pop()
                si.on_wait.append(ws[-1])
                for k, w in enumerate(ws[:-1]):
                    nd = mybir.InstDrain(
                        name=f"{inst.name}-wsplit{k}", ins=[], outs=[]
                    )
                    nd.engine = inst.engine
                    nd.sync_info = mybir.SyncInfo(on_wait=[w], on_update=[])
                    insts.insert(i, nd)
                    nc.inst_map[nd.name] = nd
                    i += 1
            i += 1


def _build_bass():
    nc = bass.Bass(trn_type="TRN2")

    blob = nc.dram_tensor("blob", [BLOB_N, 1], BF16, kind="ExternalInput")
    meta = nc.dram_tensor("meta", [META_N, 1], I32, kind="ExternalInput")
    out_dr = nc.dram_tensor("out", [RPC, W], BF16, kind="ExternalOutput")
    out_flat = out_dr[:].rearrange("a b -> (a b)").unsqueeze(1)

    def bslice(off, n, p):
        """blob[off:off+n] as a [p, n/p] tile AP (row-major fill)."""
        return blob[off:off + n, 0:1].rearrange("(p c) u -> p (c u)", p=p)

    with tile.TileContext(nc) as tc:
        with (
            tc.tile_pool(name="const", bufs=1) as constp,
            tc.tile_pool(name="big", bufs=5) as big_p,
            tc.tile_pool(name="u1", bufs=3) as u1_p,
            tc.tile_pool(name="q", bufs=4) as q_p,
            tc.tile_pool(name="w", bufs=6) as w_p,
            tc.tile_pool(name="rm", bufs=4) as rm_p,
            tc.tile_pool(name="stick", bufs=1) as stick_p,
            tc.tile_pool(name="ps1", bufs=2, space="PSUM") as ps1_p,
            tc.tile_pool(name="ps320", bufs=3, space="PSUM") as ps320_p,
            tc.tile_pool(name="dscr", bufs=1, space="DRAM") as dram_p,
        ):
            # ---- constants / small inputs
            nl_t = constp.tile([HL, SPC * WL], BF16)      # (120, 1280)
            nc.sync.dma_start(out=nl_t[:], in_=bslice(NL_OFF, NL_N, HL))
            avt_t = constp.tile([HL, H], BF16)            # (120, 480)
            nc.sync.dma_start(out=avt_t[:], in_=bslice(AVT_OFF, AVT_N, HL))
            aht_t0 = constp.tile([80, W], BF16)           # AhT rows 0:80
            aht_t1 = constp.tile([80, W], BF16)           # AhT rows 80:160
            nc.sync.dma_start(out=aht_t0[:], in_=bslice(AHT_OFF, 80 * W, 80))
            nc.sync.dma_start(out=aht_t1[:], in_=bslice(AHT_OFF + 80 * W, 80 * W, 80))
            fbv_t = stick_p.tile([SPC, 1], BF16)
            nc.sync.dma_start(out=fbv_t[:], in_=bslice(FBV_OFF, SPC, SPC))

            identf = constp.tile([ICH, ICH], F32)
            from concourse.masks import make_identity
            make_identity(nc, identf[:])
            ident = constp.tile([ICH, ICH], BF16)
            nc.vector.tensor_copy(ident[:], identf[:])

            colidx_i = constp.tile([128, W], I32)
            nc.gpsimd.iota(colidx_i[:], pattern=[[1, W]], base=0, channel_multiplier=0)
            colidx = constp.tile([128, W], F32)
            nc.vector.tensor_copy(colidx[:], colidx_i[:])

            # stick meta
            nch = N_PROW // 128
            aidx_t = stick_p.tile([SPC, 1], I32)
            nc.sync.dma_start(out=aidx_t[:], in_=meta[M_AIDX:M_AIDX + SPC, :])
            mt2 = lambda off: meta[off:off + N_PROW, 0:1].rearrange(
                "(c p) u -> p (c u)", c=nch
            )
            prow_t = stick_p.tile([128, nch], I32)
            nc.sync.dma_start(out=prow_t[:], in_=mt2(M_PROW))
            sprow_t = stick_p.tile([128, nch], I32)
            nc.sync.dma_start(out=sprow_t[:], in_=mt2(M_SPROW))
            pxlo_i = stick_p.tile([128, nch], I32)
            nc.sync.dma_start(out=pxlo_i[:], in_=mt2(M_PXLO))
            pxhi_i = stick_p.tile([128, nch], I32)
            nc.sync.dma_start(out=pxhi_i[:], in_=mt2(M_PXHI))
            pxlo_t = stick_p.tile([128, nch], F32)
            nc.vector.tensor_copy(pxlo_t[:], pxlo_i[:])
            pxhi_t = stick_p.tile([128, nch], F32)
            nc.vector.tensor_copy(pxhi_t[:], pxhi_i[:])

            out_dmas = []
            pending_out = []

            def flush_out():
                while pending_out:
                    qq, rr0, hh = pending_out.pop(0)
                    dma = nc.scalar.dma_start(
                        out=out_dr[rr0:rr0 + H, :].rearrange(
                            "(p g r) j -> p g r j", g=2, r=2
                        )[:, hh],
                        in_=qq[:, 1280 * hh:1280 * hh + 1280].rearrange(
                            "p (r j) -> p r j", r=2
                        ),
                    )
                    out_dmas.append(dma)

            for s in range(SPC):
                r0 = s * H
                # sample s inputs, split so d0 (which gates the PE stage)
                # lands first
                CW = IC * W                   # 2560 bf16 cols per tensor
                PCOLS = SAMP // ICH           # 8960 bf16 cols per partition
                big = big_p.tile([ICH, PCOLS], BF16)
                blob_s = blob[s * SAMP:(s + 1) * SAMP, 0:1].rearrange(
                    "(p c) u -> p (c u)", p=ICH
                )
                nc.sync.dma_start(out=big[:, 0:CW], in_=blob_s[:, 0:CW])
                nc.sync.dma_start(out=big[:, CW:PCOLS], in_=blob_s[:, CW:PCOLS])
                d0 = big[:, 0:CW]
                # z gate ships as fp8e5 (sign-exact), packed in bf16 slots
                dus = big[:, CW:CW + CW // 2].bitcast(FP8)
                rus = big[:, CW + CW // 2:2 * CW + CW // 2]
                rv = big[:, 2 * CW + CW // 2:3 * CW + CW // 2]

                # ---- stage 1: vertical upsample U1T = N^T @ Av^T
                u1 = []
                for c in range(2):
                    ps1 = ps1_p.tile([80, H], F32)
                    nc.tensor.matmul(
                        ps1[:],
                        lhsT=nl_t[:, s * WL + 80 * c:s * WL + 80 * (c + 1)],
                        rhs=avt_t[:], start=True, stop=True,
                    )
                    uc = u1_p.tile([80, H], BF16, tag=f"u1c{c}")
                    nc.scalar.copy(uc[:], ps1[:])
                    u1.append(uc)
                u1r = [u[:].rearrange("c (i r) -> c r i", r=IC) for u in u1]

                # ---- stage 2: horizontal upsample + d0 accumulate -> clip
                q = q_p.tile([ICH, IC * W], BF16)
                rm = rm_p.tile([ICH, IC * W], U8, tag="rm")
                for rho in range(4):
                    rb = rho * W
                    psA = ps320_p.tile([ICH, 320], F32, tag="psA")
                    psB = ps320_p.tile([ICH, 320], F32, tag="psB")
                    # weight-grouped matmul order: consecutive matmuls share
                    # the stationary tensor
                    for wt, rhsA, rhsB, fst, lst in (
                        (u1r[0][:, rho, :], aht_t0[:, 0:320],
                         aht_t0[:, 320:640], True, False),
                        (u1r[1][:, rho, :], aht_t1[:, 0:320],
                         aht_t1[:, 320:640], False, False),
                        (ident[:], d0[:, rb:rb + 320],
                         d0[:, rb + 320:rb + 640], False, True),
                    ):
                        nc.tensor.matmul(psA[:], lhsT=wt, rhs=rhsA,
                                         start=fst, stop=lst)
                        nc.tensor.matmul(psB[:], lhsT=wt, rhs=rhsB,
                                         start=fst, stop=lst)
                    # clip01, split across ACT (relu(1-relu(1-s))) and
                    # DVE (dual-op min/max) to balance engine load
                    w = w_p.tile([ICH, 320], F32, tag="w")
                    nc.scalar.activation(
                        out=w[:], in_=psA[:],
                        func=mybir.ActivationFunctionType.Relu,
                        scale=-1.0, bias=1.0,
                    )
                    nc.scalar.activation(
                        out=q[:, rb:rb + 320], in_=w[:],
                        func=mybir.ActivationFunctionType.Relu,
                        scale=-1.0, bias=1.0,
                    )
                    nc.vector.tensor_scalar(
                        out=q[:, rb + 320:rb + 640], in0=psB[:],
                        scalar1=0.0, scalar2=1.0, op0=OP.max, op1=OP.min,
                    )
                    if rho % 2 == 0:
                        continue
                    # ---- per-half mask chain on (120, 1280)
                    qsl = slice(rb - W, rb + W)
                    # dus ships as z = min(depth*2^30 - 1, dropout_u - P):
                    # z >= 0  <=>  (depth > 0) AND (dropout_u >= P), so one
                    # stt applies both the validity and dropout gates
                    nc.vector.scalar_tensor_tensor(
                        out=q[:, qsl], in0=dus[:, qsl], scalar=0.0,
                        in1=q[:, qsl], op0=OP.is_ge, op1=OP.mult,
                    )
                    nc.vector.tensor_scalar(
                        out=rm[:, qsl], in0=rus[:, qsl], scalar1=0.0,
                        scalar2=None, op0=OP.is_lt,
                    )
                    nc.vector.copy_predicated(
                        out=q[:, qsl], mask=rm[:, qsl], data=rv[:, qsl]
                    )
                    # defer each half's out DMA so the ACT sequencer never
                    # stalls waiting for this sample's DVE chain
                    flush_out()
                    pending_out.append((q, r0, rho // 2))
            flush_out()

            # ---- stick tail ----
            # column masks depend only on meta: compute them up front so the
            # post-output tail is just gather -> paint -> scatter
            cms, gs, vrows = [], [], []
            for ch in range(nch):
                cm1 = stick_p.tile([128, W], U8, tag=f"cm1{ch}")
                nc.vector.tensor_scalar(
                    out=cm1[:], in0=colidx[:], scalar1=pxlo_t[:, ch:ch + 1],
                    scalar2=None, op0=OP.is_ge,
                )
                cm = stick_p.tile([128, W], U8, tag=f"cm{ch}")
                nc.vector.scalar_tensor_tensor(
                    out=cm[:], in0=colidx[:], scalar=pxhi_t[:, ch:ch + 1],
                    in1=cm1[:], op0=OP.is_lt, op1=OP.mult,
                )
                cms.append(cm)
                g = stick_p.tile([128, W], BF16, tag=f"g{ch}")
                nc.vector.memset(g[:], 0.0)
                gs.append(g)

            anch = stick_p.tile([SPC, 1], BF16)
            ga = nc.gpsimd.indirect_dma_start(
                out=anch[:], out_offset=None, in_=out_flat,
                in_offset=bass.IndirectOffsetOnAxis(ap=aidx_t[:, :1], axis=0),
            )
            for d in out_dmas:
                tile.add_dep_helper(ga.ins, d.ins)
            # painted-row gathers are independent of the anchor/value chain
            for ch in range(nch):
                gr = nc.gpsimd.indirect_dma_start(
                    out=gs[ch][:], out_offset=None, in_=out_dr[:],
                    in_offset=bass.IndirectOffsetOnAxis(
                        ap=prow_t[:, ch:ch + 1], axis=0
                    ),
                    bounds_check=RPC - 1, oob_is_err=False,
                )
                for d in out_dmas:
                    tile.add_dep_helper(gr.ins, d.ins)
            m8 = stick_p.tile([SPC, 1], U8)
            nc.vector.tensor_scalar(
                out=m8[:], in0=anch[:], scalar1=0.0, scalar2=None, op0=OP.is_gt
            )
            val = stick_p.tile([SPC, 1], BF16)
            nc.vector.tensor_copy(val[:], fbv_t[:])
            nc.vector.copy_predicated(out=val[:], mask=m8[:], data=anch[:])
            vscr = dram_p.tile([SPC, 1], BF16)
            nc.sync.dma_start(out=vscr[:], in_=val[:])

            for ch in range(nch):
                vrow = stick_p.tile([128, 1], BF16, tag=f"vrow{ch}")
                nc.gpsimd.indirect_dma_start(
                    out=vrow[:], out_offset=None, in_=vscr[:],
                    in_offset=bass.IndirectOffsetOnAxis(
                        ap=sprow_t[:, ch:ch + 1], axis=0
                    ),
                )
                vrows.append(vrow)
            for ch in range(nch):
                nc.vector.copy_predicated(
                    out=gs[ch][:], mask=cms[ch][:],
                    data=vrows[ch][:].to_broadcast([128, W]),
                )
            for ch in range(nch):
                nc.gpsimd.indirect_dma_start(
                    out=out_dr[:],
                    out_offset=bass.IndirectOffsetOnAxis(
                        ap=prow_t[:, ch:ch + 1], axis=0
                    ),
                    in_=gs[ch][:], in_offset=None,
                    bounds_check=RPC - 1, oob_is_err=False,
                )

    _split_multiwaits(nc)
    return nc


def _stick_params(stick_len, stick_width, stick_y, stick_x, horiz_u, stick_u):
    """Vectorized reference stick geometry (ints, host side)."""
    length = stick_len.astype(np.int64) + 1
    width = stick_width.astype(np.int64) + 1
    horiz = horiz_u > 0.5
    span_h = np.where(horiz, width, length)
    span_w = np.where(horiz, length, width)
    y = np.clip(stick_y.astype(np.int64), 0, np.maximum(H - span_h, 1) - 1)
    x = np.clip(stick_x.astype(np.int64), 0, np.maximum(W - span_w, 1) - 1)
    stick_on = stick_u < np.float32(P_STICK * H * W)
    return y, x, span_h, span_w, stick_on


def _to_bf16(a):
    return np.asarray(a, dtype=np.float32).astype(ml_dtypes.bfloat16)


_NC_CACHE = []


def kernel(**inputs):
    depth = np.asarray(inputs["depth"], dtype=np.float32)
    noise_lo = np.asarray(inputs["noise_lo"], dtype=np.float32)
    dropout_u = np.asarray(inputs["dropout_u"], dtype=np.float32)
    random_u = np.asarray(inputs["random_u"], dtype=np.float32)
    random_vals = np.asarray(inputs["random_vals"], dtype=np.float32)
    stick_u = np.asarray(inputs["stick_u"], dtype=np.float32)
    horiz_u = np.asarray(inputs["horiz_u"], dtype=np.float32)
    fallback_vals = np.asarray(inputs["fallback_vals"], dtype=np.float32)
    stick_len = np.asarray(inputs["stick_len"])
    stick_width = np.asarray(inputs["stick_width"])
    stick_y = np.asarray(inputs["stick_y"])
    stick_x = np.asarray(inputs["stick_x"])

    T32 = np.float32(P_DROPOUT)
    avt = _to_bf16(_upsample_matrix(H, HL).T)         # (120, 480)
    aht = _to_bf16(_upsample_matrix(W, WL).T)         # (160, 640)

    y, x, span_h, span_w, stick_on = _stick_params(
        stick_len, stick_width, stick_y, stick_x, horiz_u, stick_u
    )

    depth_b = _to_bf16(depth).reshape(B, ICH, IC * W)
    # z < 0 iff depth == 0 or dropout_u < P (branch-free sign encoding;
    # nonzero uniform f32 depths are >= 2^-23, so depth*2^30 - 1 > 0).
    # Shipped as fp8e5m2 after scaling by 2^40: |z| >= ~2^-32 so the scaled
    # magnitude is >= 2^8, far above fp8 underflow; overflow saturates to
    # +-inf with the sign intact. The device only tests z >= 0.
    z8_b = (
        np.minimum(depth * np.float32(2.0 ** 30) - np.float32(1.0),
                   dropout_u - T32) * np.float32(2.0 ** 40)
    ).astype(ml_dtypes.float8_e5m2).view(np.uint8).reshape(B, ICH, IC * W)
    rus_b = _to_bf16(random_u - T32).reshape(B, ICH, IC * W)
    rv_b = _to_bf16(random_vals).reshape(B, ICH, IC * W)
    # noise_lo * sigma, transposed per core to [120, SPC*160]
    nl_s = (noise_lo[:, 0] * np.float32(NOISE_SIGMA)).astype(ml_dtypes.bfloat16)
    fbv_b = _to_bf16(fallback_vals)

    in_maps = []
    for k in range(N_CORES):
        s0 = k * SPC
        sl = slice(s0, s0 + SPC)
        blob = np.empty((BLOB_N, 1), ml_dtypes.bfloat16)
        # partition-major byte layout per [sample][partition]:
        # depth 5120B | z8 2560B | rus 5120B | rv 5120B
        bigv = blob[:NL_OFF, 0].view(np.uint8).reshape(SPC, ICH, 17920)
        bigv[:, :, 0:5120] = depth_b[sl].view(np.uint8)
        bigv[:, :, 5120:7680] = z8_b[sl]
        bigv[:, :, 7680:12800] = rus_b[sl].view(np.uint8)
        bigv[:, :, 12800:17920] = rv_b[sl].view(np.uint8)
        # [SPC, 120, 160] -> [120, SPC, 160]
        blob[NL_OFF:NL_OFF + NL_N, 0] = np.ascontiguousarray(
            nl_s[sl].transpose(1, 0, 2)
        ).reshape(-1)
        blob[AVT_OFF:AVT_OFF + AVT_N, 0] = avt.reshape(-1)
        blob[AHT_OFF:AHT_OFF + AHT_N, 0] = aht.reshape(-1)
        blob[FBV_OFF:FBV_OFF + SPC, 0] = fbv_b[sl]

        metav = np.zeros((META_N, 1), np.int32)
        prow = metav[M_PROW:M_PROW + N_PROW, 0]
        prow[:] = PAD_IDX
        sprow = metav[M_SPROW:M_SPROW + N_PROW, 0]
        pxlo = metav[M_PXLO:M_PXLO + N_PROW, 0]
        pxhi = metav[M_PXHI:M_PXHI + N_PROW, 0]
        aidx = metav[M_AIDX:M_AIDX + SPC, 0]
        n = 0
        for s in range(SPC):
            b = s0 + s
            aidx[s] = (s * H + y[b]) * W + x[b]
            if not stick_on[b]:
                continue
            for r in range(int(span_h[b])):
                prow[n] = s * H + y[b] + r
                sprow[n] = s
                pxlo[n] = x[b]
                pxhi[n] = x[b] + span_w[b]
                n += 1
        in_maps.append({"blob": blob, "meta": metav})

    if not _NC_CACHE:
        _NC_CACHE.append(_build_bass())
    nc = _NC_CACHE[0]
    res = run_bass_kernel_spmd(nc, in_maps, core_ids=list(range(N_CORES)))
    out = np.empty((B, 1, H, W), np.float32)
    for k in range(N_CORES):
        out[k * SPC:(k + 1) * SPC, 0] = (
            res.results[k]["out"].astype(np.float32).reshape(SPC, H, W)
        )
    return out


# revision 30
# speedup vs baseline: 1.4270x; 1.1523x over previous
"""DepthAugmentation Trainium2 kernel (v2: bf16 + packed buffers).

Reference pipeline (B=64, H=480, W=640, all f32):
  1. noise = bilinear_upsample(noise_lo * sigma, 4x)   (half-pixel centers)
     depth = clip(depth + noise * (depth > 0), 0, 1)
  2. depth *= (dropout_u >= P_DROPOUT)
  3. depth = where(random_u < P_RANDOM, random_vals, depth)
  4. per-sample stick rectangle painted with the (post-step-3) anchor value.

Sharding: pure data parallel, 8 samples per core on 8 NeuronCores.

v2 changes vs v1:
  - All big tensors ship as bf16 (half the HBM + host-relay traffic).
    Threshold tensors ship SHIFTED: dus = bf16(dropout_u - P), compared
    vs 0 on device. Sign is preserved exactly through the f32->bf16
    rounding (|du - P| >= ~2^-32 >> bf16 min normal), so the dropout /
    random masks are bit-identical to the f32 reference's.
  - All inputs packed into ONE bf16 blob + ONE i32 meta tensor per core
    (the per-exec host-relay cost scales with buffer count).
  - Output is bf16, upcast to f32 on the host (max abs err ~2^-9, well
    inside the 2e-2 gate).
  - Single bf16 matmul path for the 4x bilinear upsample (no hi/lo
    split): upsample weights {.125,.375,.625,.875,1} are exact in bf16,
    sigma is folded into noise_lo on the host.
"""

import numpy as np
import ml_dtypes

import concourse.bass as bass
import concourse.tile as tile
from concourse import mybir
from concourse.bass_utils import run_bass_kernel_spmd

F32 = mybir.dt.float32
BF16 = mybir.dt.bfloat16
FP8 = mybir.dt.float8e5
I32 = mybir.dt.int32
U8 = mybir.dt.uint8
OP = mybir.AluOpType

NOISE_SIGMA = 0.005
P_DROPOUT = 0.003125
P_RANDOM = 0.003125
P_STICK = 0.00025

B, H, W = 64, 480, 640
HL, WL = 120, 160          # noise_lo spatial dims
N_CORES = 8
SPC = B // N_CORES         # samples per core
RPC = SPC * H              # output rows per core block (3840)
IC = 4                     # rows per partition group
ICH = H // IC              # 120
PAD_IDX = 1 << 30          # OOB sentinel for indirect DMA padding
N_PROW = 256               # painted-row slots (2 chunks of 128)

# ---- blob layout (bf16 elements) ----
PIX = H * W                       # 307200
# per sample: depth(bf16)|z8(fp8e5, half the slots)|rus(bf16)|rv(bf16),
# partition-major so each partition line is one contiguous 17920B run
SAMP = 3 * PIX + PIX // 2         # 1075200
NL_OFF = SPC * SAMP               # 8601600
NL_N = HL * SPC * WL              # 153600 ([120, 8*160] layout)
AVT_OFF = NL_OFF + NL_N
AVT_N = HL * H                    # 57600
AHT_OFF = AVT_OFF + AVT_N
AHT_N = WL * W                    # 102400
FBV_OFF = AHT_OFF + AHT_N
BLOB_N = FBV_OFF + SPC            # 10144008

# ---- meta layout (i32 elements) ----
M_AIDX = 0                        # [SPC] anchor element index into out
M_PROW = M_AIDX + SPC             # [N_PROW] painted row gather index
M_SPROW = M_PROW + N_PROW         # [N_PROW] sample of each painted row
M_PXLO = M_SPROW + N_PROW         # [N_PROW] stick col start
M_PXHI = M_PXLO + N_PROW          # [N_PROW] stick col end
META_N = M_PXHI + N_PROW          # 1032


def _upsample_matrix(n_out, n_in):
    """Bilinear upsample matrix, half-pixel centers, edge clamp."""
    A = np.zeros((n_out, n_in), dtype=np.float64)
    scale = n_in / n_out
    for i in range(n_out):
        src = (i + 0.5) * scale - 0.5
        k0 = int(np.floor(src))
        f = src - k0
        A[i, min(max(k0, 0), n_in - 1)] += 1.0 - f
        A[i, min(max(k0 + 1, 0), n_in - 1)] += f
    return A.astype(np.float32)


def _split_multiwaits(nc):
    """This container's walrus build only accepts ONE sync-wait command per
    CTRL instruction; Tile's epilogue drain carries several. Hoist extra
    waits onto single-wait drains inserted just before the offender."""
    for b in nc.m.functions[0].blocks:
        insts = b.instructions
        i = 0
        while i < len(insts):
            inst = insts[i]
            si = inst.sync_info
            if si is not None and si.on_wait is not None and len(si.on_wait) > 1:
                ws = list(si.on_wait)
                while si.on_wait:
                    si.on_wait.pop()
                si.on_wait.append(ws[-1])
                for k, w in enumerate(ws[:-1]):
                    nd = mybir.InstDrain(
                        name=f"{inst.name}-wsplit{k}", ins=[], outs=[]
                    )
                    nd.engine = inst.engine
                    nd.sync_info = mybir.SyncInfo(on_wait=[w], on_update=[])
                    insts.insert(i, nd)
                    nc.inst_map[nd.name] = nd
                    i += 1
            i += 1


def _build_bass():
    nc = bass.Bass(trn_type="TRN2")

    blob = nc.dram_tensor("blob", [BLOB_N, 1], BF16, kind="ExternalInput")
    meta = nc.dram_tensor("meta", [META_N, 1], I32, kind="ExternalInput")
    out_dr = nc.dram_tensor("out", [RPC, W], BF16, kind="ExternalOutput")
    out_flat = out_dr[:].rearrange("a b -> (a b)").unsqueeze(1)

    def bslice(off, n, p):
        """blob[off:off+n] as a [p, n/p] tile AP (row-major fill)."""
        return blob[off:off + n, 0:1].rearrange("(p c) u -> p (c u)", p=p)

    with tile.TileContext(nc) as tc:
        with (
            tc.tile_pool(name="const", bufs=1) as constp,
            tc.tile_pool(name="big", bufs=5) as big_p,
            tc.tile_pool(name="u1", bufs=3) as u1_p,
            tc.tile_pool(name="q", bufs=4) as q_p,
            tc.tile_pool(name="w", bufs=6) as w_p,
            tc.tile_pool(name="rm", bufs=4) as rm_p,
            tc.tile_pool(name="stick", bufs=1) as stick_p,
            tc.tile_pool(name="ps1", bufs=2, space="PSUM") as ps1_p,
            tc.tile_pool(name="ps320", bufs=3, space="PSUM") as ps320_p,
            tc.tile_pool(name="dscr", bufs=1, space="DRAM") as dram_p,
        ):
            # ---- constants / small inputs
            nl_t = constp.tile([HL, SPC * WL], BF16)      # (120, 1280)
            nc.sync.dma_start(out=nl_t[:], in_=bslice(NL_OFF, NL_N, HL))
            avt_t = constp.tile([HL, H], BF16)            # (120, 480)
            nc.sync.dma_start(out=avt_t[:], in_=bslice(AVT_OFF, AVT_N, HL))
            aht_t0 = constp.tile([80, W], BF16)           # AhT rows 0:80
            aht_t1 = constp.tile([80, W], BF16)           # AhT rows 80:160
            nc.sync.dma_start(out=aht_t0[:], in_=bslice(AHT_OFF, 80 * W, 80))
            nc.sync.dma_start(out=aht_t1[:], in_=bslice(AHT_OFF + 80 * W, 80 * W, 80))
            fbv_t = stick_p.tile([SPC, 1], BF16)
            nc.sync.dma_start(out=fbv_t[:], in_=bslice(FBV_OFF, SPC, SPC))

            identf = constp.tile([ICH, ICH], F32)
            from concourse.masks import make_identity
            make_identity(nc, identf[:])
            ident = constp.tile([ICH, ICH], BF16)
            nc.vector.tensor_copy(ident[:], identf[:])

            colidx_i = constp.tile([128, W], I32)
            nc.gpsimd.iota(colidx_i[:], pattern=[[1, W]], base=0, channel_multiplier=0)
            colidx = constp.tile([128, W], F32)
            nc.vector.tensor_copy(colidx[:], colidx_i[:])

            # stick meta
            nch = N_PROW // 128
            aidx_t = stick_p.tile([SPC, 1], I32)
            nc.sync.dma_start(out=aidx_t[:], in_=meta[M_AIDX:M_AIDX + SPC, :])
            mt2 = lambda off: meta[off:off + N_PROW, 0:1].rearrange(
                "(c p) u -> p (c u)", c=nch
            )
            prow_t = stick_p.tile([128, nch], I32)
            nc.sync.dma_start(out=prow_t[:], in_=mt2(M_PROW))
            sprow_t = stick_p.tile([128, nch], I32)
            nc.sync.dma_start(out=sprow_t[:], in_=mt2(M_SPROW))
            pxlo_i = stick_p.tile([128, nch], I32)
            nc.sync.dma_start(out=pxlo_i[:], in_=mt2(M_PXLO))
            pxhi_i = stick_p.tile([128, nch], I32)
            nc.sync.dma_start(out=pxhi_i[:], in_=mt2(M_PXHI))
            pxlo_t = stick_p.tile([128, nch], F32)
            nc.vector.tensor_copy(pxlo_t[:], pxlo_i[:])
            pxhi_t = stick_p.tile([128, nch], F32)
            nc.vector.tensor_copy(pxhi_t[:], pxhi_i[:])

            out_dmas = []
            pending_out = []

            def flush_out():
                while pending_out:
                    qq, rr0, hh = pending_out.pop(0)
                    dma = nc.scalar.dma_start(
                        out=out_dr[rr0:rr0 + H, :].rearrange(
                            "(p g r) j -> p g r j", g=2, r=2
                        )[:, hh],
                        in_=qq[:, 1280 * hh:1280 * hh + 1280].rearrange(
                            "p (r j) -> p r j", r=2
                        ),
                    )
                    out_dmas.append(dma)

            for s in range(SPC):
                r0 = s * H
                # sample s inputs, split so d0 (which gates the PE stage)
                # lands first
                CW = IC * W                   # 2560 bf16 cols per tensor
                PCOLS = SAMP // ICH           # 8960 bf16 cols per partition
                big = big_p.tile([ICH, PCOLS], BF16)
                blob_s = blob[s * SAMP:(s + 1) * SAMP, 0:1].rearrange(
                    "(p c) u -> p (c u)", p=ICH
                )
                nc.sync.dma_start(out=big[:, 0:CW], in_=blob_s[:, 0:CW])
                nc.sync.dma_start(out=big[:, CW:PCOLS], in_=blob_s[:, CW:PCOLS])
                d0 = big[:, 0:CW]
                # z gate ships as fp8e5 (sign-exact), packed in bf16 slots
                dus = big[:, CW:CW + CW // 2].bitcast(FP8)
                rus = big[:, CW + CW // 2:2 * CW + CW // 2]
                rv = big[:, 2 * CW + CW // 2:3 * CW + CW // 2]

                # ---- stage 1: vertical upsample U1T = N^T @ Av^T
                u1 = []
                for c in range(2):
                    ps1 = ps1_p.tile([80, H], F32)
                    nc.tensor.matmul(
                        ps1[:],
                        lhsT=nl_t[:, s * WL + 80 * c:s * WL + 80 * (c + 1)],
                        rhs=avt_t[:], start=True, stop=True,
                    )
                    uc = u1_p.tile([80, H], BF16, tag=f"u1c{c}")
                    nc.scalar.copy(uc[:], ps1[:])
                    u1.append(uc)
                u1r = [u[:].rearrange("c (i r) -> c r i", r=IC) for u in u1]

                # ---- stage 2: horizontal upsample + d0 accumulate -> clip
                q = q_p.tile([ICH, IC * W], BF16)
                rm = rm_p.tile([ICH, IC * W], U8, tag="rm")
                for rho in range(4):
                    rb = rho * W
                    psA = ps320_p.tile([ICH, 320], F32, tag="psA")
                    psB = ps320_p.tile([ICH, 320], F32, tag="psB")
                    # weight-grouped matmul order: consecutive matmuls share
                    # the stationary tensor. psB skips the identity/d0
                    # accumulate — its DVE evacuation adds d0 instead,
                    # trimming the PE stream.
                    for wt, rhsA, rhsB, fst, lstA, lstB in (
                        (u1r[0][:, rho, :], aht_t0[:, 0:320],
                         aht_t0[:, 320:640], True, False, False),
                        (u1r[1][:, rho, :], aht_t1[:, 0:320],
                         aht_t1[:, 320:640], False, False, True),
                    ):
                        nc.tensor.matmul(psA[:], lhsT=wt, rhs=rhsA,
                                         start=fst, stop=lstA)
                        nc.tensor.matmul(psB[:], lhsT=wt, rhs=rhsB,
                                         start=fst, stop=lstB)
                    nc.tensor.matmul(psA[:], lhsT=ident[:],
                                     rhs=d0[:, rb:rb + 320],
                                     start=False, stop=True)
                    # clip01, split across ACT (relu(1-relu(1-s))) and
                    # DVE (dual-op min/max) to balance engine load
                    w = w_p.tile([ICH, 320], F32, tag="w")
                    nc.scalar.activation(
                        out=w[:], in_=psA[:],
                        func=mybir.ActivationFunctionType.Relu,
                        scale=-1.0, bias=1.0,
                    )
                    nc.scalar.activation(
                        out=q[:, rb:rb + 320], in_=w[:],
                        func=mybir.ActivationFunctionType.Relu,
                        scale=-1.0, bias=1.0,
                    )
                    t = w_p.tile([ICH, 320], F32, tag="t")
                    nc.vector.scalar_tensor_tensor(
                        out=t[:], in0=psB[:], scalar=0.0,
                        in1=d0[:, rb + 320:rb + 640], op0=OP.add, op1=OP.add,
                    )
                    nc.vector.tensor_scalar(
                        out=q[:, rb + 320:rb + 640], in0=t[:],
                        scalar1=0.0, scalar2=1.0, op0=OP.max, op1=OP.min,
                    )
                    if rho % 2 == 0:
                        continue
                    # ---- per-half mask chain on (120, 1280)
                    qsl = slice(rb - W, rb + W)
                    # dus ships as z = min(depth*2^30 - 1, dropout_u - P):
                    # z >= 0  <=>  (depth > 0) AND (dropout_u >= P), so one
                    # stt applies both the validity and dropout gates
                    nc.vector.scalar_tensor_tensor(
                        out=q[:, qsl], in0=dus[:, qsl], scalar=0.0,
                        in1=q[:, qsl], op0=OP.is_ge, op1=OP.mult,
                    )
                    nc.vector.tensor_scalar(
                        out=rm[:, qsl], in0=rus[:, qsl], scalar1=0.0,
                        scalar2=None, op0=OP.is_lt,
                    )
                    nc.vector.copy_predicated(
                        out=q[:, qsl], mask=rm[:, qsl], data=rv[:, qsl]
                    )
                    # defer each half's out DMA so the ACT sequencer never
                    # stalls waiting for this sample's DVE chain
                    flush_out()
                    pending_out.append((q, r0, rho // 2))
            flush_out()

            # ---- stick tail ----
            # column masks depend only on meta: compute them up front so the
            # post-output tail is just gather -> paint -> scatter
            cms, gs, vrows = [], [], []
            for ch in range(nch):
                cm1 = stick_p.tile([128, W], U8, tag=f"cm1{ch}")
                nc.vector.tensor_scalar(
                    out=cm1[:], in0=colidx[:], scalar1=pxlo_t[:, ch:ch + 1],
                    scalar2=None, op0=OP.is_ge,
                )
                cm = stick_p.tile([128, W], U8, tag=f"cm{ch}")
                nc.vector.scalar_tensor_tensor(
                    out=cm[:], in0=colidx[:], scalar=pxhi_t[:, ch:ch + 1],
                    in1=cm1[:], op0=OP.is_lt, op1=OP.mult,
                )
                cms.append(cm)
                g = stick_p.tile([128, W], BF16, tag=f"g{ch}")
                nc.vector.memset(g[:], 0.0)
                gs.append(g)

            anch = stick_p.tile([SPC, 1], BF16)
            ga = nc.gpsimd.indirect_dma_start(
                out=anch[:], out_offset=None, in_=out_flat,
                in_offset=bass.IndirectOffsetOnAxis(ap=aidx_t[:, :1], axis=0),
            )
            for d in out_dmas:
                tile.add_dep_helper(ga.ins, d.ins)
            # painted-row gathers are independent of the anchor/value chain
            for ch in range(nch):
                gr = nc.gpsimd.indirect_dma_start(
                    out=gs[ch][:], out_offset=None, in_=out_dr[:],
                    in_offset=bass.IndirectOffsetOnAxis(
                        ap=prow_t[:, ch:ch + 1], axis=0
                    ),
                    bounds_check=RPC - 1, oob_is_err=False,
                )
                for d in out_dmas:
                    tile.add_dep_helper(gr.ins, d.ins)
            m8 = stick_p.tile([SPC, 1], U8)
            nc.vector.tensor_scalar(
                out=m8[:], in0=anch[:], scalar1=0.0, scalar2=None, op0=OP.is_gt
            )
            val = stick_p.tile([SPC, 1], BF16)
            nc.vector.tensor_copy(val[:], fbv_t[:])
            nc.vector.copy_predicated(out=val[:], mask=m8[:], data=anch[:])
            vscr = dram_p.tile([SPC, 1], BF16)
            nc.sync.dma_start(out=vscr[:], in_=val[:])

            for ch in range(nch):
                vrow = stick_p.tile([128, 1], BF16, tag=f"vrow{ch}")
                nc.gpsimd.indirect_dma_start(
                    out=vrow[:], out_offset=None, in_=vscr[:],
                    in_offset=bass.IndirectOffsetOnAxis(
                        ap=sprow_t[:, ch:ch + 1], axis=0
                    ),
                )
                vrows.append(vrow)
            for ch in range(nch):
                nc.vector.copy_predicated(
                    out=gs[ch][:], mask=cms[ch][:],
                    data=vrows[ch][:].to_broadcast([128, W]),
                )
            for ch in range(nch):
                nc.gpsimd.indirect_dma_start(
                    out=out_dr[:],
                    out_offset=bass.IndirectOffsetOnAxis(
                        ap=prow_t[:, ch:ch + 1], axis=0
                    ),
                    in_=gs[ch][:], in_offset=None,
                    bounds_check=RPC - 1, oob_is_err=False,
                )

    _split_multiwaits(nc)
    return nc


def _stick_params(stick_len, stick_width, stick_y, stick_x, horiz_u, stick_u):
    """Vectorized reference stick geometry (ints, host side)."""
    length = stick_len.astype(np.int64) + 1
    width = stick_width.astype(np.int64) + 1
    horiz = horiz_u > 0.5
    span_h = np.where(horiz, width, length)
    span_w = np.where(horiz, length, width)
    y = np.clip(stick_y.astype(np.int64), 0, np.maximum(H - span_h, 1) - 1)
    x = np.clip(stick_x.astype(np.int64), 0, np.maximum(W - span_w, 1) - 1)
    stick_on = stick_u < np.float32(P_STICK * H * W)
    return y, x, span_h, span_w, stick_on


def _to_bf16(a):
    return np.asarray(a, dtype=np.float32).astype(ml_dtypes.bfloat16)


_NC_CACHE = []


def kernel(**inputs):
    depth = np.asarray(inputs["depth"], dtype=np.float32)
    noise_lo = np.asarray(inputs["noise_lo"], dtype=np.float32)
    dropout_u = np.asarray(inputs["dropout_u"], dtype=np.float32)
    random_u = np.asarray(inputs["random_u"], dtype=np.float32)
    random_vals = np.asarray(inputs["random_vals"], dtype=np.float32)
    stick_u = np.asarray(inputs["stick_u"], dtype=np.float32)
    horiz_u = np.asarray(inputs["horiz_u"], dtype=np.float32)
    fallback_vals = np.asarray(inputs["fallback_vals"], dtype=np.float32)
    stick_len = np.asarray(inputs["stick_len"])
    stick_width = np.asarray(inputs["stick_width"])
    stick_y = np.asarray(inputs["stick_y"])
    stick_x = np.asarray(inputs["stick_x"])

    T32 = np.float32(P_DROPOUT)
    avt = _to_bf16(_upsample_matrix(H, HL).T)         # (120, 480)
    aht = _to_bf16(_upsample_matrix(W, WL).T)         # (160, 640)

    y, x, span_h, span_w, stick_on = _stick_params(
        stick_len, stick_width, stick_y, stick_x, horiz_u, stick_u
    )

    depth_b = _to_bf16(depth).reshape(B, ICH, IC * W)
    # z < 0 iff depth == 0 or dropout_u < P (branch-free sign encoding;
    # nonzero uniform f32 depths are >= 2^-23, so depth*2^30 - 1 > 0).
    # Shipped as fp8e5m2 after scaling by 2^40: |z| >= ~2^-32 so the scaled
    # magnitude is >= 2^8, far above fp8 underflow; overflow saturates to
    # +-inf with the sign intact. The device only tests z >= 0.
    z8_b = (
        np.minimum(depth * np.float32(2.0 ** 30) - np.float32(1.0),
                   dropout_u - T32) * np.float32(2.0 ** 40)
    ).astype(ml_dtypes.float8_e5m2).view(np.uint8).reshape(B, ICH, IC * W)
    rus_b = _to_bf16(random_u - T32).reshape(B, ICH, IC * W)
    rv_b = _to_bf16(random_vals).reshape(B, ICH, IC * W)
    # noise_lo * sigma, transposed per core to [120, SPC*160]
    nl_s = (noise_lo[:, 0] * np.float32(NOISE_SIGMA)).astype(ml_dtypes.bfloat16)
    fbv_b = _to_bf16(fallback_vals)

    in_maps = []
    for k in range(N_CORES):
        s0 = k * SPC
        sl = slice(s0, s0 + SPC)
        blob = np.empty((BLOB_N, 1), ml_dtypes.bfloat16)
        # partition-major byte layout per [sample][partition]:
        # depth 5120B | z8 2560B | rus 5120B | rv 5120B
        bigv = blob[:NL_OFF, 0].view(np.uint8).reshape(SPC, ICH, 17920)
        bigv[:, :, 0:5120] = depth_b[sl].view(np.uint8)
        bigv[:, :, 5120:7680] = z8_b[sl]
        bigv[:, :, 7680:12800] = rus_b[sl].view(np.uint8)
        bigv[:, :, 12800:17920] = rv_b[sl].view(np.uint8)
        # [SPC, 120, 160] -> [120, SPC, 160]
        blob[NL_OFF:NL_OFF + NL_N, 0] = np.ascontiguousarray(
            nl_s[sl].transpose(1, 0, 2)
        ).reshape(-1)
        blob[AVT_OFF:AVT_OFF + AVT_N, 0] = avt.reshape(-1)
        blob[AHT_OFF:AHT_OFF + AHT_N, 0] = aht.reshape(-1)
        blob[FBV_OFF:FBV_OFF + SPC, 0] = fbv_b[sl]

        metav = np.zeros((META_N, 1), np.int32)
        prow = metav[M_PROW:M_PROW + N_PROW, 0]
        prow[:] = PAD_IDX
        sprow = metav[M_SPROW:M_SPROW + N_PROW, 0]
        pxlo = metav[M_PXLO:M_PXLO + N_PROW, 0]
        pxhi = metav[M_PXHI:M_PXHI + N_PROW, 0]
        aidx = metav[M_AIDX:M_AIDX + SPC, 0]
        n = 0
        for s in range(SPC):
            b = s0 + s
            aidx[s] = (s * H + y[b]) * W + x[b]
            if not stick_on[b]:
                continue
            for r in range(int(span_h[b])):
                prow[n] = s * H + y[b] + r
                sprow[n] = s
                pxlo[n] = x[b]
                pxhi[n] = x[b] + span_w[b]
                n += 1
        in_maps.append({"blob": blob, "meta": metav})

    if not _NC_CACHE:
        _NC_CACHE.append(_build_bass())
    nc = _NC_CACHE[0]
    res = run_bass_kernel_spmd(nc, in_maps, core_ids=list(range(N_CORES)))
    out = np.empty((B, 1, H, W), np.float32)
    for k in range(N_CORES):
        out[k * SPC:(k + 1) * SPC, 0] = (
            res.results[k]["out"].astype(np.float32).reshape(SPC, H, W)
        )
    return out


# revision 37
# speedup vs baseline: 1.4553x; 1.0199x over previous
"""DepthAugmentation Trainium2 kernel (v2: bf16 + packed buffers).

Reference pipeline (B=64, H=480, W=640, all f32):
  1. noise = bilinear_upsample(noise_lo * sigma, 4x)   (half-pixel centers)
     depth = clip(depth + noise * (depth > 0), 0, 1)
  2. depth *= (dropout_u >= P_DROPOUT)
  3. depth = where(random_u < P_RANDOM, random_vals, depth)
  4. per-sample stick rectangle painted with the (post-step-3) anchor value.

Sharding: pure data parallel, 8 samples per core on 8 NeuronCores.

v2 changes vs v1:
  - All big tensors ship as bf16 (half the HBM + host-relay traffic).
    Threshold tensors ship SHIFTED: dus = bf16(dropout_u - P), compared
    vs 0 on device. Sign is preserved exactly through the f32->bf16
    rounding (|du - P| >= ~2^-32 >> bf16 min normal), so the dropout /
    random masks are bit-identical to the f32 reference's.
  - All inputs packed into ONE bf16 blob + ONE i32 meta tensor per core
    (the per-exec host-relay cost scales with buffer count).
  - Output is bf16, upcast to f32 on the host (max abs err ~2^-9, well
    inside the 2e-2 gate).
  - Single bf16 matmul path for the 4x bilinear upsample (no hi/lo
    split): upsample weights {.125,.375,.625,.875,1} are exact in bf16,
    sigma is folded into noise_lo on the host.
"""

import numpy as np
import ml_dtypes

import concourse.bass as bass
import concourse.tile as tile
from concourse import mybir
from concourse.bass_utils import run_bass_kernel_spmd

F32 = mybir.dt.float32
BF16 = mybir.dt.bfloat16
FP8 = mybir.dt.float8e5
I32 = mybir.dt.int32
U8 = mybir.dt.uint8
OP = mybir.AluOpType

NOISE_SIGMA = 0.005
P_DROPOUT = 0.003125
P_RANDOM = 0.003125
P_STICK = 0.00025

B, H, W = 64, 480, 640
HL, WL = 120, 160          # noise_lo spatial dims
N_CORES = 8
SPC = B // N_CORES         # samples per core
RPC = SPC * H              # output rows per core block (3840)
IC = 4                     # rows per partition group
ICH = H // IC              # 120
PAD_IDX = 1 << 30          # OOB sentinel for indirect DMA padding
N_PROW = 256               # painted-row slots (2 chunks of 128)

# ---- blob layout (bf16 elements) ----
PIX = H * W                       # 307200
# per sample: depth(bf16)|z8(fp8e5, half the slots)|rus(bf16)|rv(bf16),
# partition-major so each partition line is one contiguous 17920B run
SAMP = 3 * PIX + PIX // 2         # 1075200
NL_OFF = SPC * SAMP               # 8601600
NL_N = HL * SPC * WL              # 153600 ([120, 8*160] layout)
AVT_OFF = NL_OFF + NL_N
AVT_N = HL * H                    # 57600
AHT_OFF = AVT_OFF + AVT_N
AHT_N = WL * W                    # 102400
FBV_OFF = AHT_OFF + AHT_N
SEL_OFF = FBV_OFF + SPC           # [SPC, N_PROW] one-hot row->sample select
SEL_N = SPC * N_PROW              # 2048
BLOB_N = SEL_OFF + SEL_N

# ---- meta layout (i32 elements) ----
M_AIDX = 0                        # [SPC] anchor element index into out
M_PROW = M_AIDX + SPC             # [N_PROW] painted row gather index
M_SPROW = M_PROW + N_PROW         # [N_PROW] sample of each painted row
M_PXLO = M_SPROW + N_PROW         # [N_PROW] stick col start
M_PXHI = M_PXLO + N_PROW          # [N_PROW] stick col end
META_N = M_PXHI + N_PROW          # 1032


def _upsample_matrix(n_out, n_in):
    """Bilinear upsample matrix, half-pixel centers, edge clamp."""
    A = np.zeros((n_out, n_in), dtype=np.float64)
    scale = n_in / n_out
    for i in range(n_out):
        src = (i + 0.5) * scale - 0.5
        k0 = int(np.floor(src))
        f = src - k0
        A[i, min(max(k0, 0), n_in - 1)] += 1.0 - f
        A[i, min(max(k0 + 1, 0), n_in - 1)] += f
    return A.astype(np.float32)


def _split_multiwaits(nc):
    """This container's walrus build only accepts ONE sync-wait command per
    CTRL instruction; Tile's epilogue drain carries several. Hoist extra
    waits onto single-wait drains inserted just before the offender."""
    for b in nc.m.functions[0].blocks:
        insts = b.instructions
        i = 0
        while i < len(insts):
            inst = insts[i]
            si = inst.sync_info
            if si is not None and si.on_wait is not None and len(si.on_wait) > 1:
                ws = list(si.on_wait)
                while si.on_wait:
                    si.on_wait.pop()
                si.on_wait.append(ws[-1])
                for k, w in enumerate(ws[:-1]):
                    nd = mybir.InstDrain(
                        name=f"{inst.name}-wsplit{k}", ins=[], outs=[]
                    )
                    nd.engine = inst.engine
                    nd.sync_info = mybir.SyncInfo(on_wait=[w], on_update=[])
                    insts.insert(i, nd)
                    nc.inst_map[nd.name] = nd
                    i += 1
            i += 1


def _build_bass():
    nc = bass.Bass(trn_type="TRN2")

    blob = nc.dram_tensor("blob", [BLOB_N, 1], BF16, kind="ExternalInput")
    meta = nc.dram_tensor("meta", [META_N, 1], I32, kind="ExternalInput")
    out_dr = nc.dram_tensor("out", [RPC, W], BF16, kind="ExternalOutput")
    out_flat = out_dr[:].rearrange("a b -> (a b)").unsqueeze(1)

    def bslice(off, n, p):
        """blob[off:off+n] as a [p, n/p] tile AP (row-major fill)."""
        return blob[off:off + n, 0:1].rearrange("(p c) u -> p (c u)", p=p)

    with tile.TileContext(nc) as tc:
        with (
            tc.tile_pool(name="const", bufs=1) as constp,
            tc.tile_pool(name="big", bufs=5) as big_p,
            tc.tile_pool(name="u1", bufs=3) as u1_p,
            tc.tile_pool(name="q", bufs=4) as q_p,
            tc.tile_pool(name="w", bufs=6) as w_p,
            tc.tile_pool(name="rm", bufs=4) as rm_p,
            tc.tile_pool(name="stick", bufs=1) as stick_p,
            tc.tile_pool(name="ps1", bufs=1, space="PSUM") as ps1_p,
            tc.tile_pool(name="psv", bufs=1, space="PSUM") as psv_p,
            tc.tile_pool(name="ps320", bufs=3, space="PSUM") as ps320_p,
            tc.tile_pool(name="dscr", bufs=1, space="DRAM") as dram_p,
        ):
            # ---- constants / small inputs
            nl_t = constp.tile([HL, SPC * WL], BF16)      # (120, 1280)
            nc.sync.dma_start(out=nl_t[:], in_=bslice(NL_OFF, NL_N, HL))
            avt_t = constp.tile([HL, H], BF16)            # (120, 480)
            nc.sync.dma_start(out=avt_t[:], in_=bslice(AVT_OFF, AVT_N, HL))
            aht_t0 = constp.tile([80, W], BF16)           # AhT rows 0:80
            aht_t1 = constp.tile([80, W], BF16)           # AhT rows 80:160
            nc.sync.dma_start(out=aht_t0[:], in_=bslice(AHT_OFF, 80 * W, 80))
            nc.sync.dma_start(out=aht_t1[:], in_=bslice(AHT_OFF + 80 * W, 80 * W, 80))
            fbv_t = stick_p.tile([SPC, 1], BF16)
            nc.sync.dma_start(out=fbv_t[:], in_=bslice(FBV_OFF, SPC, SPC))
            sel_t = stick_p.tile([SPC, N_PROW], BF16)
            nc.sync.dma_start(out=sel_t[:], in_=bslice(SEL_OFF, SEL_N, SPC))

            identf = constp.tile([ICH, ICH], F32)
            from concourse.masks import make_identity
            make_identity(nc, identf[:])
            ident = constp.tile([ICH, ICH], BF16)
            nc.vector.tensor_copy(ident[:], identf[:])

            colidx_i = constp.tile([128, W], I32)
            nc.gpsimd.iota(colidx_i[:], pattern=[[1, W]], base=0, channel_multiplier=0)
            colidx = constp.tile([128, W], F32)
            nc.vector.tensor_copy(colidx[:], colidx_i[:])

            # stick meta
            nch = N_PROW // 128
            aidx_t = stick_p.tile([SPC, 1], I32)
            nc.sync.dma_start(out=aidx_t[:], in_=meta[M_AIDX:M_AIDX + SPC, :])
            mt2 = lambda off: meta[off:off + N_PROW, 0:1].rearrange(
                "(c p) u -> p (c u)", c=nch
            )
            prow_t = stick_p.tile([128, nch], I32)
            nc.sync.dma_start(out=prow_t[:], in_=mt2(M_PROW))
            sprow_t = stick_p.tile([128, nch], I32)
            nc.sync.dma_start(out=sprow_t[:], in_=mt2(M_SPROW))
            pxlo_i = stick_p.tile([128, nch], I32)
            nc.sync.dma_start(out=pxlo_i[:], in_=mt2(M_PXLO))
            pxhi_i = stick_p.tile([128, nch], I32)
            nc.sync.dma_start(out=pxhi_i[:], in_=mt2(M_PXHI))
            pxlo_t = stick_p.tile([128, nch], F32)
            nc.vector.tensor_copy(pxlo_t[:], pxlo_i[:])
            pxhi_t = stick_p.tile([128, nch], F32)
            nc.vector.tensor_copy(pxhi_t[:], pxhi_i[:])

            out_dmas = []
            pending_out = []

            def flush_out():
                while pending_out:
                    qq, rr0, hh = pending_out.pop(0)
                    dma = nc.scalar.dma_start(
                        out=out_dr[rr0:rr0 + H, :].rearrange(
                            "(p g r) j -> p g r j", g=2, r=2
                        )[:, hh],
                        in_=qq[:, 1280 * hh:1280 * hh + 1280].rearrange(
                            "p (r j) -> p r j", r=2
                        ),
                    )
                    out_dmas.append(dma)

            for s in range(SPC):
                r0 = s * H
                # sample s inputs, split so d0 (which gates the PE stage)
                # lands first
                CW = IC * W                   # 2560 bf16 cols per tensor
                PCOLS = SAMP // ICH           # 8960 bf16 cols per partition
                big = big_p.tile([ICH, PCOLS], BF16)
                blob_s = blob[s * SAMP:(s + 1) * SAMP, 0:1].rearrange(
                    "(p c) u -> p (c u)", p=ICH
                )
                nc.sync.dma_start(out=big[:, 0:CW], in_=blob_s[:, 0:CW])
                nc.sync.dma_start(out=big[:, CW:PCOLS], in_=blob_s[:, CW:PCOLS])
                d0 = big[:, 0:CW]
                # z gate ships as fp8e5 (sign-exact), packed in bf16 slots
                dus = big[:, CW:CW + CW // 2].bitcast(FP8)
                rus = big[:, CW + CW // 2:2 * CW + CW // 2]
                rv = big[:, 2 * CW + CW // 2:3 * CW + CW // 2]

                # ---- stage 1: vertical upsample U1T = N^T @ Av^T
                u1 = []
                for c in range(2):
                    ps1 = ps1_p.tile([80, H], F32)
                    nc.tensor.matmul(
                        ps1[:],
                        lhsT=nl_t[:, s * WL + 80 * c:s * WL + 80 * (c + 1)],
                        rhs=avt_t[:], start=True, stop=True,
                    )
                    uc = u1_p.tile([80, H], BF16, tag=f"u1c{c}")
                    nc.scalar.copy(uc[:], ps1[:])
                    u1.append(uc)
                u1r = [u[:].rearrange("c (i r) -> c r i", r=IC) for u in u1]

                # ---- stage 2: horizontal upsample + d0 accumulate -> clip
                q = q_p.tile([ICH, IC * W], BF16)
                rm = rm_p.tile([ICH, IC * W], U8, tag="rm")
                for rho in range(4):
                    rb = rho * W
                    psA = ps320_p.tile([ICH, 320], F32, tag="psA")
                    psB = ps320_p.tile([ICH, 320], F32, tag="psB")
                    # weight-grouped matmul order: consecutive matmuls share
                    # the stationary tensor
                    for wt, rhsA, rhsB, fst, lst in (
                        (u1r[0][:, rho, :], aht_t0[:, 0:320],
                         aht_t0[:, 320:640], True, False),
                        (u1r[1][:, rho, :], aht_t1[:, 0:320],
                         aht_t1[:, 320:640], False, False),
                        (ident[:], d0[:, rb:rb + 320],
                         d0[:, rb + 320:rb + 640], False, True),
                    ):
                        nc.tensor.matmul(psA[:], lhsT=wt, rhs=rhsA,
                                         start=fst, stop=lst)
                        nc.tensor.matmul(psB[:], lhsT=wt, rhs=rhsB,
                                         start=fst, stop=lst)
                    # clip01, split across ACT (relu(1-relu(1-s))) and
                    # DVE (dual-op min/max) to balance engine load
                    w = w_p.tile([ICH, 320], F32, tag="w")
                    nc.scalar.activation(
                        out=w[:], in_=psA[:],
                        func=mybir.ActivationFunctionType.Relu,
                        scale=-1.0, bias=1.0,
                    )
                    nc.scalar.activation(
                        out=q[:, rb:rb + 320], in_=w[:],
                        func=mybir.ActivationFunctionType.Relu,
                        scale=-1.0, bias=1.0,
                    )
                    nc.vector.tensor_scalar(
                        out=q[:, rb + 320:rb + 640], in0=psB[:],
                        scalar1=0.0, scalar2=1.0, op0=OP.max, op1=OP.min,
                    )
                    if rho % 2 == 0:
                        continue
                    # ---- per-half mask chain on (120, 1280)
                    qsl = slice(rb - W, rb + W)
                    # dus ships as z = min(depth*2^30 - 1, dropout_u - P):
                    # z >= 0  <=>  (depth > 0) AND (dropout_u >= P), so one
                    # stt applies both the validity and dropout gates
                    nc.vector.scalar_tensor_tensor(
                        out=q[:, qsl], in0=dus[:, qsl], scalar=0.0,
                        in1=q[:, qsl], op0=OP.is_ge, op1=OP.mult,
                    )
                    nc.vector.tensor_scalar(
                        out=rm[:, qsl], in0=rus[:, qsl], scalar1=0.0,
                        scalar2=None, op0=OP.is_lt,
                    )
                    nc.vector.copy_predicated(
                        out=q[:, qsl], mask=rm[:, qsl], data=rv[:, qsl]
                    )
                    # defer each half's out DMA so the ACT sequencer never
                    # stalls waiting for this sample's DVE chain
                    flush_out()
                    pending_out.append((q, r0, rho // 2))
            flush_out()

            # ---- stick tail ----
            # column masks depend only on meta: compute them up front so the
            # post-output tail is just gather -> paint -> scatter
            cms, gs, vrows = [], [], []
            for ch in range(nch):
                cm1 = stick_p.tile([128, W], U8, tag=f"cm1{ch}")
                nc.vector.tensor_scalar(
                    out=cm1[:], in0=colidx[:], scalar1=pxlo_t[:, ch:ch + 1],
                    scalar2=None, op0=OP.is_ge,
                )
                cm = stick_p.tile([128, W], U8, tag=f"cm{ch}")
                nc.vector.scalar_tensor_tensor(
                    out=cm[:], in0=colidx[:], scalar=pxhi_t[:, ch:ch + 1],
                    in1=cm1[:], op0=OP.is_lt, op1=OP.mult,
                )
                cms.append(cm)
                g = stick_p.tile([128, W], BF16, tag=f"g{ch}")
                nc.vector.memset(g[:], 0.0)
                gs.append(g)

            anch = stick_p.tile([SPC, 1], BF16)
            ga = nc.gpsimd.indirect_dma_start(
                out=anch[:], out_offset=None, in_=out_flat,
                in_offset=bass.IndirectOffsetOnAxis(ap=aidx_t[:, :1], axis=0),
            )
            for d in out_dmas:
                tile.add_dep_helper(ga.ins, d.ins)
            # painted-row gathers are independent of the anchor/value chain
            for ch in range(nch):
                gr = nc.gpsimd.indirect_dma_start(
                    out=gs[ch][:], out_offset=None, in_=out_dr[:],
                    in_offset=bass.IndirectOffsetOnAxis(
                        ap=prow_t[:, ch:ch + 1], axis=0
                    ),
                    bounds_check=RPC - 1, oob_is_err=False,
                )
                for d in out_dmas:
                    tile.add_dep_helper(gr.ins, d.ins)
            m8 = stick_p.tile([SPC, 1], U8)
            nc.vector.tensor_scalar(
                out=m8[:], in0=anch[:], scalar1=0.0, scalar2=None, op0=OP.is_gt
            )
            val = stick_p.tile([SPC, 1], BF16)
            nc.vector.tensor_copy(val[:], fbv_t[:])
            nc.vector.copy_predicated(out=val[:], mask=m8[:], data=anch[:])

            # distribute val to painted-row slots with a one-hot matmul
            # (PE is idle by now) instead of a DRAM scratch round-trip
            for ch in range(nch):
                psv = psv_p.tile([128, 1], F32)
                nc.tensor.matmul(
                    psv[:], lhsT=sel_t[:, ch * 128:(ch + 1) * 128],
                    rhs=val[:], start=True, stop=True,
                )
                vrow = stick_p.tile([128, 1], BF16, tag=f"vrow{ch}")
                nc.vector.tensor_copy(vrow[:], psv[:])
                vrows.append(vrow)
            for ch in range(nch):
                nc.vector.copy_predicated(
                    out=gs[ch][:], mask=cms[ch][:],
                    data=vrows[ch][:].to_broadcast([128, W]),
                )
            for ch in range(nch):
                nc.gpsimd.indirect_dma_start(
                    out=out_dr[:],
                    out_offset=bass.IndirectOffsetOnAxis(
                        ap=prow_t[:, ch:ch + 1], axis=0
                    ),
                    in_=gs[ch][:], in_offset=None,
                    bounds_check=RPC - 1, oob_is_err=False,
                )

    _split_multiwaits(nc)
    return nc


def _stick_params(stick_len, stick_width, stick_y, stick_x, horiz_u, stick_u):
    """Vectorized reference stick geometry (ints, host side)."""
    length = stick_len.astype(np.int64) + 1
    width = stick_width.astype(np.int64) + 1
    horiz = horiz_u > 0.5
    span_h = np.where(horiz, width, length)
    span_w = np.where(horiz, length, width)
    y = np.clip(stick_y.astype(np.int64), 0, np.maximum(H - span_h, 1) - 1)
    x = np.clip(stick_x.astype(np.int64), 0, np.maximum(W - span_w, 1) - 1)
    stick_on = stick_u < np.float32(P_STICK * H * W)
    return y, x, span_h, span_w, stick_on


def _to_bf16(a):
    return np.asarray(a, dtype=np.float32).astype(ml_dtypes.bfloat16)


_NC_CACHE = []


def kernel(**inputs):
    depth = np.asarray(inputs["depth"], dtype=np.float32)
    noise_lo = np.asarray(inputs["noise_lo"], dtype=np.float32)
    dropout_u = np.asarray(inputs["dropout_u"], dtype=np.float32)
    random_u = np.asarray(inputs["random_u"], dtype=np.float32)
    random_vals = np.asarray(inputs["random_vals"], dtype=np.float32)
    stick_u = np.asarray(inputs["stick_u"], dtype=np.float32)
    horiz_u = np.asarray(inputs["horiz_u"], dtype=np.float32)
    fallback_vals = np.asarray(inputs["fallback_vals"], dtype=np.float32)
    stick_len = np.asarray(inputs["stick_len"])
    stick_width = np.asarray(inputs["stick_width"])
    stick_y = np.asarray(inputs["stick_y"])
    stick_x = np.asarray(inputs["stick_x"])

    T32 = np.float32(P_DROPOUT)
    avt = _to_bf16(_upsample_matrix(H, HL).T)         # (120, 480)
    aht = _to_bf16(_upsample_matrix(W, WL).T)         # (160, 640)

    y, x, span_h, span_w, stick_on = _stick_params(
        stick_len, stick_width, stick_y, stick_x, horiz_u, stick_u
    )

    depth_b = _to_bf16(depth).reshape(B, ICH, IC * W)
    # z < 0 iff depth == 0 or dropout_u < P (branch-free sign encoding;
    # nonzero uniform f32 depths are >= 2^-23, so depth*2^30 - 1 > 0).
    # Shipped as fp8e5m2 after scaling by 2^40: |z| >= ~2^-32 so the scaled
    # magnitude is >= 2^8, far above fp8 underflow; overflow saturates to
    # +-inf with the sign intact. The device only tests z >= 0.
    z8_b = (
        np.minimum(depth * np.float32(2.0 ** 30) - np.float32(1.0),
                   dropout_u - T32) * np.float32(2.0 ** 40)
    ).astype(ml_dtypes.float8_e5m2).view(np.uint8).reshape(B, ICH, IC * W)
    rus_b = _to_bf16(random_u - T32).reshape(B, ICH, IC * W)
    rv_b = _to_bf16(random_vals).reshape(B, ICH, IC * W)
    # noise_lo * sigma, transposed per core to [120, SPC*160]
    nl_s = (noise_lo[:, 0] * np.float32(NOISE_SIGMA)).astype(ml_dtypes.bfloat16)
    fbv_b = _to_bf16(fallback_vals)

    in_maps = []
    for k in range(N_CORES):
        s0 = k * SPC
        sl = slice(s0, s0 + SPC)
        blob = np.empty((BLOB_N, 1), ml_dtypes.bfloat16)
        # partition-major byte layout per [sample][partition]:
        # depth 5120B | z8 2560B | rus 5120B | rv 5120B
        bigv = blob[:NL_OFF, 0].view(np.uint8).reshape(SPC, ICH, 17920)
        bigv[:, :, 0:5120] = depth_b[sl].view(np.uint8)
        bigv[:, :, 5120:7680] = z8_b[sl]
        bigv[:, :, 7680:12800] = rus_b[sl].view(np.uint8)
        bigv[:, :, 12800:17920] = rv_b[sl].view(np.uint8)
        # [SPC, 120, 160] -> [120, SPC, 160]
        blob[NL_OFF:NL_OFF + NL_N, 0] = np.ascontiguousarray(
            nl_s[sl].transpose(1, 0, 2)
        ).reshape(-1)
        blob[AVT_OFF:AVT_OFF + AVT_N, 0] = avt.reshape(-1)
        blob[AHT_OFF:AHT_OFF + AHT_N, 0] = aht.reshape(-1)
        blob[FBV_OFF:FBV_OFF + SPC, 0] = fbv_b[sl]
        sel = np.zeros((SPC, N_PROW), ml_dtypes.bfloat16)

        metav = np.zeros((META_N, 1), np.int32)
        prow = metav[M_PROW:M_PROW + N_PROW, 0]
        prow[:] = PAD_IDX
        sprow = metav[M_SPROW:M_SPROW + N_PROW, 0]
        pxlo = metav[M_PXLO:M_PXLO + N_PROW, 0]
        pxhi = metav[M_PXHI:M_PXHI + N_PROW, 0]
        aidx = metav[M_AIDX:M_AIDX + SPC, 0]
        n = 0
        for s in range(SPC):
            b = s0 + s
            aidx[s] = (s * H + y[b]) * W + x[b]
            if not stick_on[b]:
                continue
            for r in range(int(span_h[b])):
                prow[n] = s * H + y[b] + r
                sprow[n] = s
                sel[s, n] = 1
                pxlo[n] = x[b]
                pxhi[n] = x[b] + span_w[b]
                n += 1
        blob[SEL_OFF:SEL_OFF + SEL_N, 0] = sel.reshape(-1)
        in_maps.append({"blob": blob, "meta": metav})

    if not _NC_CACHE:
        _NC_CACHE.append(_build_bass())
    nc = _NC_CACHE[0]
    res = run_bass_kernel_spmd(nc, in_maps, core_ids=list(range(N_CORES)))
    out = np.empty((B, 1, H, W), np.float32)
    for k in range(N_CORES):
        out[k * SPC:(k + 1) * SPC, 0] = (
            res.results[k]["out"].astype(np.float32).reshape(SPC, H, W)
        )
    return out
